# revision 1
# baseline (speedup 1.0000x reference)
"""Trainium2 Bass kernel for ClassicalReconstructionHydraSSMCore.

Quantum statevector simulation: batch 8192, 10 qubits, three circuits
(forward/backward/diagonal), combine + normalize + Pauli X/Y/Z measure.

Sharding: pure data parallel over batch across 8 cores (1024 each).
Per-core layout: batch on partitions (8 tiles of 128), state on free dim
(1024 re + 1024 im fp32 per circuit).
"""

import numpy as np

import concourse.bass as bass
import concourse.tile as tile
from concourse import bacc, mybir

F32 = mybir.dt.float32
AOT = mybir.AluOpType
ACTF = mybir.ActivationFunctionType


def _register_axpby():
    """Runtime-register a custom DVE op: out = in0*s0 + in1*s1."""
    import concourse.dve_ops as dve_ops
    from concourse.dve_spec import Spec, Src0, Src1, C0, C1, lower
    from concourse.dve_spec import _has_src1 as has_src1
    from concourse.dve_uop import DveOpSpec

    name = "AXPBY9_ANT"
    for op in dve_ops.OPS:
        if op.name == name:
            return op
    spec = Spec(
        body=Src0 * C0 + Src1 * C1,
        reference=lambda in0, in1, s0, s1, imm2: in0 * s0 + in1 * s1,
    )
    row = dve_ops._CUSTOM_DVE_ROW_BASE + len(dve_ops.OPS)
    assert row < 0x20
    dve_ops._SUB_OPCODE_FOR_NAME[name] = row
    shas = {}
    for ver in ("v3", "v4"):
        s = DveOpSpec(
            name=name, opcode=row, uops=lower(spec, ver=ver), rd1_en=has_src1(spec)
        )
        shas[ver] = s.sha(ver)
    op = dve_ops.DveOp(name, spec, subdim=False, uops_sha=shas)
    dve_ops.OPS.append(op)
    dve_ops.CUSTOM_DVE_SPECS[name] = spec
    return op


AXPBY = _register_axpby()

NQ = 10
DIM = 1 << NQ          # 1024
HD = DIM // 2          # 512
P = 128
N_CORES = 8
B_CORE = 1024
NT = B_CORE // P       # 8 tiles per core
PI_2 = float(np.pi / 2)

# circuit ids
FWD, BWD, DIAG = 0, 1, 2


def _wire(c, g):
    """u-coefficient column group g -> wire, per circuit."""
    return (NQ - 1 - g) if c == BWD else g


def _ring_gates(c, L):
    """Time-ordered entangler list [(ctrl, tgt, col)] for circuit c, layer L.

    col is the column inside the (128, 300) trig tiles.
    """
    base = 100 * c + 50 * L
    out = []
    if c in (FWD, DIAG):
        for k in range(NQ):       # ring1: CRX(i, i+1), i ascending
            out.append((k, (k + 1) % NQ, base + 30 + k))
        for k in range(NQ):       # ring2: CRX(i, i-1), i descending
            i = NQ - 1 - k
            out.append((i, (i - 1) % NQ, base + 40 + k))
    else:  # BWD
        for k in range(NQ):       # ring1: CRX(i, i-1), i descending
            i = NQ - 1 - k
            out.append((i, (i - 1) % NQ, base + 30 + k))
        for k in range(NQ):       # ring2: CRX(i, i+1), i ascending
            out.append((k, (k + 1) % NQ, base + 40 + k))
    return out


def _wire_views(plane, w):
    """(a0, a1) views of a (128, 1024) plane AP for wire w; shape (128, O, I)."""
    inner = 1 << (NQ - 1 - w)
    outer = HD // inner
    v = plane.rearrange("p (o t i) -> p o t i", o=outer, t=2, i=inner)
    return v[:, :, 0, :], v[:, :, 1, :]


def _tview(tp, w):
    inner = 1 << (NQ - 1 - w)
    outer = HD // inner
    return tp[:, 0:HD].rearrange("p (o i) -> p o i", o=outer, i=inner)


def _qviews2(plane, ctrl, tgt):
    """ctrl=1 quarters (tgt=0, tgt=1) of a (128,1024) plane; <=2 free dims."""
    hi, lo = min(ctrl, tgt), max(ctrl, tgt)
    if lo - hi == 1:
        a = 1 << hi
        z = 1 << (NQ - 2 - hi)
        v = plane.rearrange("p (a x y z) -> p a x y z", a=a, x=2, y=2, z=z)
        if ctrl < tgt:
            return v[:, :, 1, 0, :], v[:, :, 1, 1, :]
        return v[:, :, 0, 1, :], v[:, :, 1, 1, :]
    assert hi == 0 and lo == NQ - 1
    v = plane.rearrange("p (x b y) -> p x b y", x=2, b=DIM // 4, y=2)
    if ctrl == 0:
        return v[:, 1, :, 0], v[:, 1, :, 1]
    return v[:, 0, :, 1], v[:, 1, :, 1]


def _q0both(stfull, ctrl, tgt):
    """q0 (ctrl=1, tgt=0) across both planes of the (128,2048) state tile."""
    hi, lo = min(ctrl, tgt), max(ctrl, tgt)
    if lo - hi == 1:
        a = 1 << hi
        z = 1 << (NQ - 2 - hi)
        v = stfull.rearrange(
            "p (pl a x y z) -> p pl a x y z", pl=2, a=a, x=2, y=2, z=z
        )
        if ctrl < tgt:
            return v[:, :, :, 1, 0, :]
        return v[:, :, :, 0, 1, :]
    v = stfull.rearrange("p (pl x b y) -> p pl x b y", pl=2, x=2, b=DIM // 4, y=2)
    if ctrl == 0:
        return v[:, :, 1, :, 0]
    return v[:, :, 0, :, 1]


def _qtboth(stfull, ctrl, tgt, tbit):
    """ctrl=1, tgt=tbit quarter across both planes of the (128,2048) tile."""
    hi, lo = min(ctrl, tgt), max(ctrl, tgt)
    if lo - hi == 1:
        a = 1 << hi
        z = 1 << (NQ - 2 - hi)
        v = stfull.rearrange(
            "p (pl a x y z) -> p pl a x y z", pl=2, a=a, x=2, y=2, z=z
        )
        if ctrl < tgt:
            return v[:, :, :, 1, tbit, :]
        return v[:, :, :, tbit, 1, :]
    v = stfull.rearrange("p (pl x b y) -> p pl x b y", pl=2, x=2, b=DIM // 4, y=2)
    if ctrl == 0:
        return v[:, :, 1, :, tbit]
    return v[:, :, tbit, :, 1]


def _qshape(ctrl, tgt):
    hi, lo = min(ctrl, tgt), max(ctrl, tgt)
    if lo - hi == 1:
        return (1 << hi, 1 << (NQ - 2 - hi))
    return (DIM // 4,)


def _tq2(tp, off, ctrl, tgt):
    """(128, 256) slice of tp at column off, viewed to match _qviews2 shape."""
    sh = _qshape(ctrl, tgt)
    sl = tp[:, off : off + DIM // 4]
    if len(sh) == 1:
        return sl
    return sl.rearrange("p (a z) -> p a z", a=sh[0], z=sh[1])


def emit_core_kernel(nc, tc, ins, outs, n_tiles=NT):
    """Emit the full per-core kernel. ins/outs are dicts of DRAM APs."""
    ang_d = ins["input_angles"]
    par_d = [ins["forward_params"], ins["backward_params"], ins["diagonal_params"]]
    dth_d = ins["dth"]
    cf_d = ins["cf"]
    out_d = outs["out"]

    stt_ = lambda out, in0, sc, in1: nc.vector.scalar_tensor_tensor(
        out, in0, sc, in1, op0=AOT.mult, op1=AOT.add
    )
    tsd = nc.vector.tensor_scalar_mul         # ts on DVE
    tsa = nc.scalar.mul                       # ts on ACT
    ttp = nc.gpsimd.tensor_tensor             # TT on Pool
    ttd = nc.vector.tensor_tensor             # TT on DVE

    ax = lambda out, x, sx, y, sy: nc.vector._custom_dve(
        AXPBY, out=out, in0=x, in1=y, s0=sx, s1=sy
    )

    with (
        tc.tile_pool(name="const", bufs=1) as cpool,
        tc.tile_pool(name="work", bufs=2) as pool,
        tc.tile_pool(name="tmps", bufs=1) as tpool,
    ):
        cf_t = cpool.tile([P, 16], F32)
        nc.sync.dma_start(cf_t[:, 0 : cf_d.shape[1]], cf_d[:])
        pi2 = cpool.tile([P, 1], F32)
        nc.gpsimd.memset(pi2[:], PI_2)
        pi2c = pi2[:, 0:1]

        for t in range(n_tiles):
            r0, r1 = t * P, (t + 1) * P
            # ---- loads ----
            par = pool.tile([P, 300], F32, tag="par")
            for c in range(3):
                nc.sync.dma_start(par[:, 100 * c : 100 * (c + 1)], par_d[c][r0:r1, :])
            ang = pool.tile([P, NQ], F32, tag="ang")
            nc.sync.dma_start(ang[:], ang_d[r0:r1, :])
            dth = pool.tile([P, 1], F32, tag="dth")
            nc.sync.dma_start(dth[:], dth_d[r0:r1, :])

            # ---- trig ----
            # ScalarE Sin only covers [-pi, pi], so use quarter angles:
            # u = sin(h/2), w = sin(h/2 + pi/2) = cos(h/2);
            # sin(h) = 2uw, cos(h) = 1 - 2u^2.  (dth input holds 0.25*dt.)
            ch = pool.tile([P, 300], F32, tag="ch")
            sh = pool.tile([P, 300], F32, tag="sh")
            nsh = pool.tile([P, 300], F32, tag="nsh")
            trA = pool.tile([P, 100], F32, tag="trA")
            trB = pool.tile([P, 100], F32, tag="trB")

            def emit_trig(dst_s, dst_c, src, scale, scrA, scrB):
                nc.scalar.activation(dst_s, src, ACTF.Sin, scale=scale)
                nc.scalar.activation(dst_c, src, ACTF.Sin, scale=scale, bias=pi2c)
                ttp(scrA, dst_s, dst_c, op=AOT.mult)
                ttp(scrB, dst_s, dst_s, op=AOT.mult)
                nc.gpsimd.tensor_scalar_mul(dst_s, scrA, 2.0)
                nc.gpsimd.tensor_scalar(dst_c, scrB, -2.0, 1.0, op0=AOT.mult, op1=AOT.add)

            for c in range(3):
                src = par[:, 100 * c : 100 * (c + 1)]
                dst_s = sh[:, 100 * c : 100 * (c + 1)]
                dst_c = ch[:, 100 * c : 100 * (c + 1)]
                if c == DIAG:
                    emit_trig(dst_s, dst_c, src, 0.25, trA[:], trB[:])
                else:
                    emit_trig(dst_s, dst_c, src, dth[:, 0:1], trA[:], trB[:])
                    # fix CRX cols (30-49, 80-99): no dt factor
                    lx = lambda ap: ap.rearrange("p (l x) -> p l x", l=2, x=50)[:, :, 30:50]
                    emit_trig(
                        lx(dst_s), lx(dst_c), lx(src), 0.25,
                        trA[:, 0:40].rearrange("p (l x) -> p l x", l=2, x=20),
                        trB[:, 0:40].rearrange("p (l x) -> p l x", l=2, x=20),
                    )
            nc.vector.tensor_scalar_mul(nsh[:], sh[:], -1.0)

            angc = pool.tile([P, NQ], F32, tag="angc")
            angs = pool.tile([P, NQ], F32, tag="angs")
            emit_trig(angs[:], angc[:], ang[:], 0.25, trA[:, 0:NQ], trB[:, 0:NQ])
            a3c = pool.tile([P, 30], F32, tag="a3c")
            a3s = pool.tile([P, 30], F32, tag="a3s")
            nc.scalar.copy(a3c[:, 0:10], angc[:])
            nc.scalar.copy(a3c[:, 10:20], angc[:, ::-1])
            nc.scalar.copy(a3c[:, 20:30], angc[:])
            nc.scalar.copy(a3s[:, 0:10], angs[:])
            nc.scalar.copy(a3s[:, 10:20], angs[:, ::-1])
            nc.scalar.copy(a3s[:, 20:30], angs[:])

            # ---- u-coefficients per layer: p,q,nq,r,nr,s,ns (128,30) ----
            ch3 = ch[:].rearrange("p (c x) -> p c x", c=3, x=100)
            sh3 = sh[:].rearrange("p (c x) -> p c x", c=3, x=100)
            U = []  # U[L] = dict of tiles
            m1 = pool.tile([P, 30], F32, tag="m1")
            m2 = pool.tile([P, 30], F32, tag="m2")
            m3 = pool.tile([P, 30], F32, tag="m3")
            m4 = pool.tile([P, 30], F32, tag="m4")
            w1 = pool.tile([P, 30], F32, tag="w1")
            w2 = pool.tile([P, 30], F32, tag="w2")
            V = lambda tl: tl[:].rearrange("p (c g) -> p c g", c=3, g=10)
            for L in range(2):
                ca = ch3[:, :, 50 * L : 50 * L + 30 : 3]
                cb = ch3[:, :, 50 * L + 1 : 50 * L + 30 : 3]
                cg = ch3[:, :, 50 * L + 2 : 50 * L + 30 : 3]
                sa = sh3[:, :, 50 * L : 50 * L + 30 : 3]
                sb = sh3[:, :, 50 * L + 1 : 50 * L + 30 : 3]
                sg = sh3[:, :, 50 * L + 2 : 50 * L + 30 : 3]
                u = {
                    k: pool.tile([P, 30], F32, tag=f"u{k}{L}", name=f"u{k}{L}")
                    for k in ("p", "q", "nq", "r", "nr", "s", "ns")
                }
                ttp(V(m1), cb, ca, op=AOT.mult)
                ttp(V(m2), sb, sa, op=AOT.mult)
                ttp(V(m3), sb, ca, op=AOT.mult)
                ttp(V(m4), cb, sa, op=AOT.mult)
                ttp(V(w1), cg, V(m1), op=AOT.mult)
                ttp(V(w2), sg, V(m2), op=AOT.mult)
                ttp(V(u["p"]), V(w1), V(w2), op=AOT.add)
                ttp(V(w1), cg, V(m2), op=AOT.mult)
                ttp(V(w2), sg, V(m1), op=AOT.mult)
                ttp(V(u["q"]), V(w1), V(w2), op=AOT.subtract)
                ttp(V(w1), cg, V(m3), op=AOT.mult)
                ttp(V(w2), sg, V(m4), op=AOT.mult)
                ttp(V(u["nr"]), V(w1), V(w2), op=AOT.add)
                ttp(V(w1), sg, V(m3), op=AOT.mult)
                ttp(V(w2), cg, V(m4), op=AOT.mult)
                ttp(V(u["s"]), V(w1), V(w2), op=AOT.subtract)
                nc.gpsimd.tensor_scalar_mul(u["nq"][:], u["q"][:], -1.0)
                nc.gpsimd.tensor_scalar_mul(u["r"][:], u["nr"][:], -1.0)
                nc.gpsimd.tensor_scalar_mul(u["ns"][:], u["s"][:], -1.0)
                U.append(u)

            # ---- v vectors: layer-0 rotations folded into init ----
            u0 = U[0]
            v0r = pool.tile([P, 30], F32, tag="v0r")
            v0i = pool.tile([P, 30], F32, tag="v0i")
            v1r = pool.tile([P, 30], F32, tag="v1r")
            v1i = pool.tile([P, 30], F32, tag="v1i")
            nv0i = pool.tile([P, 30], F32, tag="nv0i")
            nv1i = pool.tile([P, 30], F32, tag="nv1i")
            for dst, t1, t2 in (
                (v0r, ("p", a3c), ("r", a3s)),
                (v0i, ("q", a3c), ("s", a3s)),
                (v1r, ("nr", a3c), ("p", a3s)),
                (v1i, ("s", a3c), ("nq", a3s)),
            ):
                ttp(w1[:], u0[t1[0]][:], t1[1][:], op=AOT.mult)
                ttp(w2[:], u0[t2[0]][:], t2[1][:], op=AOT.mult)
                ttp(dst[:], w1[:], w2[:], op=AOT.add)
            nc.gpsimd.tensor_scalar_mul(nv0i[:], v0i[:], -1.0)
            nc.gpsimd.tensor_scalar_mul(nv1i[:], v1i[:], -1.0)

            # ---- per-circuit state build + gates ----
            st = [pool.tile([P, 2 * DIM], F32, tag=f"st{c}", name=f"st{c}") for c in range(3)]
            tmp = [
                [tpool.tile([P, DIM], F32, tag=f"tmp{c}_{k}", name=f"tmp{c}_{k}") for k in range(4)]
                for c in range(3)
            ]
            ab = [
                [pool.tile([P, 32], F32, tag=f"ab{c}_{k}", name=f"ab{c}_{k}") for k in range(8)]
                for c in range(3)
            ]
            scr = pool.tile([P, DIM], F32, tag="scr")

            def expand(c, bufs, wires, col_of):
                """Log-doubling product build over `wires` into bufs (r,i,r2,i2)."""
                br, bi, br2, bi2 = bufs
                j0 = col_of(wires[0])
                nc.scalar.copy(br[:, 0:1], v0r[:, j0 : j0 + 1])
                nc.scalar.copy(br[:, 1:2], v1r[:, j0 : j0 + 1])
                nc.scalar.copy(bi[:, 0:1], v0i[:, j0 : j0 + 1])
                nc.scalar.copy(bi[:, 1:2], v1i[:, j0 : j0 + 1])
                width = 2
                cur_r, cur_i, oth_r, oth_i = br, bi, br2, bi2
                for w in wires[1:]:
                    j = col_of(w)
                    c0r, c0i = v0r[:, j : j + 1], v0i[:, j : j + 1]
                    c1r, c1i = v1r[:, j : j + 1], v1i[:, j : j + 1]
                    n0i, n1i = nv0i[:, j : j + 1], nv1i[:, j : j + 1]
                    old_r, old_i = cur_r[:, 0:width], cur_i[:, 0:width]
                    nw = 2 * width
                    nr_v = oth_r[:, 0:nw].rearrange("p (w t) -> p w t", w=width, t=2)
                    ni_v = oth_i[:, 0:nw].rearrange("p (w t) -> p w t", w=width, t=2)
                    tt0 = tmp[c][0][:, 0:width]
                    tt1 = tmp[c][1][:, 0:width]
                    tt2 = tmp[c][2][:, 0:width]
                    tt3 = tmp[c][3][:, 0:width]
                    tsa(tt0, old_r, c0r)
                    stt_(nr_v[:, :, 0], old_i, n0i, tt0)
                    tsa(tt1, old_r, c0i)
                    stt_(ni_v[:, :, 0], old_i, c0r, tt1)
                    tsa(tt2, old_r, c1r)
                    stt_(nr_v[:, :, 1], old_i, n1i, tt2)
                    tsa(tt3, old_r, c1i)
                    stt_(ni_v[:, :, 1], old_i, c1r, tt3)
                    cur_r, oth_r = oth_r, cur_r
                    cur_i, oth_i = oth_i, cur_i
                    width = nw
                return cur_r, cur_i

            def emit_rot(c, stt_c, w, u, j):
                sp = u["p"][:, j : j + 1]
                sq = u["q"][:, j : j + 1]
                snq = u["nq"][:, j : j + 1]
                sr = u["r"][:, j : j + 1]
                snr = u["nr"][:, j : j + 1]
                ss = u["s"][:, j : j + 1]
                sns = u["ns"][:, j : j + 1]
                re, im = stt_c[:, 0:DIM], stt_c[:, DIM : 2 * DIM]
                a0r, a1r = _wire_views(re, w)
                a0i, a1i = _wire_views(im, w)
                inner = 1 << (NQ - 1 - w)
                outer = HD // inner
                m = 2 * outer
                # plane stride (1024) == outer_count*outer_stride, so the
                # (plane, outer) dims merge: both planes in one 2-free-dim AP.
                fm = stt_c[:].rearrange("p (m t i) -> p m t i", m=m, t=2, i=inner)
                a0m = fm[:, :, 0, :]
                a1m = fm[:, :, 1, :]
                T = tmp[c]
                mv = lambda tp: tp[:].rearrange("p (m i) -> p m i", m=m, i=inner)
                hvv = lambda tp, h: tp[:, h * HD : (h + 1) * HD].rearrange(
                    "p (o i) -> p o i", o=outer, i=inner
                )
                ts_sw = (
                    nc.gpsimd.tensor_scalar_mul if w % 2 == 0 else
                    (lambda o, i, s: nc.scalar.mul(o, i, s))
                )
                # aligned groups (same scalar both planes, merged views):
                #   T0 = p*a0 + r*a1        T2 = p*a1 + nr*a0
                tsa(mv(T[0]), a0m, sp)
                stt_(mv(T[0]), a1m, sr, mv(T[0]))
                tsa(mv(T[2]), a1m, sp)
                stt_(mv(T[2]), a0m, snr, mv(T[2]))
                # swapped groups, per-plane (plane-reversed views can't merge):
                #   T1 = (nq*a0i + ns*a1i | nq*a0r + ns*a1r)
                #   T3 = (q*a1i + ns*a0i  | q*a1r + ns*a0r)
                ts_sw(hvv(T[1], 0), a0i, snq)
                stt_(hvv(T[1], 0), a1i, sns, hvv(T[1], 0))
                ts_sw(hvv(T[1], 1), a0r, snq)
                stt_(hvv(T[1], 1), a1r, sns, hvv(T[1], 1))
                ts_sw(hvv(T[3], 0), a1i, sq)
                stt_(hvv(T[3], 0), a0i, sns, hvv(T[3], 0))
                ts_sw(hvv(T[3], 1), a1r, sq)
                stt_(hvv(T[3], 1), a0r, sns, hvv(T[3], 1))
                # finals: re-plane add, im-plane subtract, in place
                ttd(a0r, hvv(T[0], 0), hvv(T[1], 0), op=AOT.add)
                ttp(a0i, hvv(T[0], 1), hvv(T[1], 1), op=AOT.subtract)
                ttd(a1r, hvv(T[2], 0), hvv(T[3], 0), op=AOT.add)
                ttp(a1i, hvv(T[2], 1), hvv(T[3], 1), op=AOT.subtract)

            def emit_crx(c, stt_c, ctrl, tgt, col):
                cc = ch[:, col : col + 1]
                ss = sh[:, col : col + 1]
                ns = nsh[:, col : col + 1]
                QT = DIM // 4
                re, im = stt_c[:, 0:DIM], stt_c[:, DIM : 2 * DIM]
                q0r, q1r = _qviews2(re, ctrl, tgt)
                q0i, q1i = _qviews2(im, ctrl, tgt)
                hi, lo = min(ctrl, tgt), max(ctrl, tgt)
                t23 = tmp[c][0]
                ee = tmp[c][1]
                if lo - hi == 1:
                    # adjacent pair: (plane, a) merges -> 2-free-dim APs
                    a = 1 << hi
                    z = 1 << (NQ - 2 - hi)
                    ma = 2 * a
                    fm = stt_c[:].rearrange(
                        "p (ma x y z) -> p ma x y z", ma=ma, x=2, y=2, z=z
                    )
                    if ctrl < tgt:
                        q0m = fm[:, :, 1, 0, :]
                        q1m = fm[:, :, 1, 1, :]
                    else:
                        q0m = fm[:, :, 0, 1, :]
                        q1m = fm[:, :, 1, 1, :]
                    pvm = lambda tp: tp[:, 0 : 2 * QT].rearrange(
                        "p (ma z) -> p ma z", ma=ma, z=z
                    )
                    hv = lambda tp, h: tp[:, h * QT : (h + 1) * QT].rearrange(
                        "p (a z) -> p a z", a=a, z=z
                    )
                    nc.gpsimd.tensor_scalar_mul(hv(t23, 0), q0i, ss)
                    nc.gpsimd.tensor_scalar_mul(hv(t23, 1), q0r, ns)
                    tsa(hv(ee, 0), q1i, ss)
                    tsa(hv(ee, 1), q1r, ns)
                    stt_(q0m, q0m, cc, pvm(ee))
                    stt_(q1m, q1m, cc, pvm(t23))
                else:
                    # wrap pair {0,9}: planes can't merge; per-plane form
                    t0 = tmp[c][0][:, 0:QT]
                    t1 = tmp[c][1][:, 0:QT]
                    t2 = tmp[c][2][:, 0:QT]
                    t3 = tmp[c][3][:, 0:QT]
                    nc.gpsimd.tensor_scalar_mul(t0, q0r, cc)
                    tsa(t1, q0i, cc)
                    nc.gpsimd.tensor_scalar_mul(t2, q0i, ss)
                    tsa(t3, q0r, ns)
                    stt_(q0r, q1i, ss, t0)
                    stt_(q0i, q1r, ns, t1)
                    stt_(q1r, q1r, cc, t2)
                    stt_(q1i, q1i, cc, t3)

            for c in range(3):
                col_of = lambda w, c=c: 10 * c + (w if c != BWD else NQ - 1 - w)
                ar, ai = expand(c, ab[c][0:4], list(range(5)), col_of)
                br_, bi_ = expand(c, ab[c][4:8], list(range(5, NQ)), col_of)
                sre = st[c][:, 0:DIM].rearrange("p (i j) -> p i j", i=32, j=32)
                sim_ = st[c][:, DIM : 2 * DIM].rearrange("p (i j) -> p i j", i=32, j=32)
                scr_v = scr[:].rearrange("p (i j) -> p i j", i=32, j=32)
                arb = ar[:].broadcast_to([P, 32, 32])
                aib = ai[:].broadcast_to([P, 32, 32])
                brb = br_[:].broadcast_to([P, 32, 32]).transpose([0, 2, 1])
                bib = bi_[:].broadcast_to([P, 32, 32]).transpose([0, 2, 1])
                ttd(sre, arb, brb, op=AOT.mult)
                ttp(scr_v, aib, bib, op=AOT.mult)
                ttd(sre, sre, scr_v, op=AOT.subtract)
                ttd(sim_, arb, bib, op=AOT.mult)
                ttp(scr_v, aib, brb, op=AOT.mult)
                ttd(sim_, sim_, scr_v, op=AOT.add)

                # layer 0 rings
                for ctrl, tgt, col in _ring_gates(c, 0):
                    emit_crx(c, st[c], ctrl, tgt, col)
                # layer 1 rotations
                for g in range(NQ):
                    emit_rot(c, st[c], _wire(c, g), U[1], 10 * c + g)
                # layer 1 rings
                for ctrl, tgt, col in _ring_gates(c, 1):
                    emit_crx(c, st[c], ctrl, tgt, col)

            # ---- combine: acc = c1*psi1 + c2*psi2 + c3*psi3 ----
            acc = pool.tile([P, 2 * DIM], F32, tag="acc")
            cfc = lambda k: cf_t[:, k : k + 1]
            # cf cols: [c1r, c1i, nc1i, c2r, c2i, nc2i, c3r, c3i, nc3i]
            tsd(acc[:], st[0][:], cfc(0))
            stt_(acc[:, 0:DIM], st[0][:, DIM : 2 * DIM], cfc(2), acc[:, 0:DIM])
            stt_(acc[:, DIM : 2 * DIM], st[0][:, 0:DIM], cfc(1), acc[:, DIM : 2 * DIM])
            for k in (1, 2):
                stt_(acc[:], st[k][:], cfc(3 * k), acc[:])
                stt_(acc[:, 0:DIM], st[k][:, DIM : 2 * DIM], cfc(3 * k + 2), acc[:, 0:DIM])
                stt_(
                    acc[:, DIM : 2 * DIM],
                    st[k][:, 0:DIM],
                    cfc(3 * k + 1),
                    acc[:, DIM : 2 * DIM],
                )

            # ---- measure ----
            crR = pool.tile([P, NQ], F32, tag="crR")
            cA = pool.tile([P, NQ], F32, tag="cA")
            cB = pool.tile([P, NQ], F32, tag="cB")
            hZ = pool.tile([P, NQ], F32, tag="hZ")
            scol = pool.tile([P, 8], F32, tag="scol")
            mscr = pool.tile([P, 2 * DIM], F32, tag="mscr")

            nc.vector.scalar_tensor_tensor(
                mscr[:], acc[:], 0.0, acc[:], op0=AOT.bypass, op1=AOT.mult,
                accum_out=scol[:, 0:1],
            )
            accr, acci = acc[:, 0:DIM], acc[:, DIM : 2 * DIM]
            for w in range(NQ):
                inner = 1 << (NQ - 1 - w)
                outer = HD // inner
                fv = acc[:].rearrange(
                    "p (m t i) -> p m t i", m=2 * outer, t=2, i=inner
                )
                p0b = fv[:, :, 0, :]
                p1b = fv[:, :, 1, :]
                ms2 = mscr[:, 0:DIM].rearrange(
                    "p (m i) -> p m i", m=2 * outer, i=inner
                )
                nc.vector.scalar_tensor_tensor(
                    ms2, p0b, 0.0, p1b, op0=AOT.bypass, op1=AOT.mult,
                    accum_out=crR[:, w : w + 1],
                )
                nc.vector.scalar_tensor_tensor(
                    ms2, p1b, 0.0, p1b, op0=AOT.bypass, op1=AOT.mult,
                    accum_out=hZ[:, w : w + 1],
                )
                p0r, _ = _wire_views(accr, w)
                p0i, _ = _wire_views(acci, w)
                _, p1r = _wire_views(accr, w)
                _, p1i = _wire_views(acci, w)
                ms1 = mscr[:, 0:HD].rearrange("p (o i) -> p o i", o=outer, i=inner)
                nc.vector.scalar_tensor_tensor(
                    ms1, p0r, 0.0, p1i, op0=AOT.bypass, op1=AOT.mult,
                    accum_out=cA[:, w : w + 1],
                )
                nc.vector.scalar_tensor_tensor(
                    ms1, p0i, 0.0, p1r, op0=AOT.bypass, op1=AOT.mult,
                    accum_out=cB[:, w : w + 1],
                )

            # inv2 = 1/(S + 1e-9); s1 = 2*inv2; s2 = -2*inv2; sz = S*inv2
            nc.vector.tensor_scalar(
                scol[:, 1:2], scol[:, 0:1], 1e-9, None, op0=AOT.add
            )
            nc.vector.reciprocal(scol[:, 2:3], scol[:, 1:2])
            nc.vector.tensor_scalar(scol[:, 3:4], scol[:, 2:3], 2.0, None, op0=AOT.mult)
            nc.vector.tensor_scalar(scol[:, 4:5], scol[:, 2:3], -2.0, None, op0=AOT.mult)
            ttd(scol[:, 5:6], scol[:, 0:1], scol[:, 2:3], op=AOT.mult)

            out30 = pool.tile([P, 30], F32, tag="out30")
            tsd(out30[:, 0:10], crR[:], scol[:, 3:4])
            ttd(w1[:, 0:10], cA[:], cB[:], op=AOT.subtract)
            tsd(out30[:, 10:20], w1[:, 0:10], scol[:, 3:4])
            szb = scol[:, 5:6].broadcast_to([P, 1, NQ])
            nc.vector.scalar_tensor_tensor(
                out30[:, 20:30].unsqueeze(1), hZ[:].unsqueeze(1), scol[:, 4:5], szb,
                op0=AOT.mult, op1=AOT.add,
            )
            nc.sync.dma_start(out_d[r0:r1, :], out30[:])


def build_nc(n_tiles=NT, b_core=None):
    if b_core is None:
        b_core = n_tiles * P
    nc = bacc.Bacc("TRN2", target_bir_lowering=False)
    ins = {
        "input_angles": nc.dram_tensor("input_angles", [b_core, NQ], F32, kind="ExternalInput")[:],
        "forward_params": nc.dram_tensor("forward_params", [b_core, 100], F32, kind="ExternalInput")[:],
        "backward_params": nc.dram_tensor("backward_params", [b_core, 100], F32, kind="ExternalInput")[:],
        "diagonal_params": nc.dram_tensor("diagonal_params", [b_core, 100], F32, kind="ExternalInput")[:],
        "dth": nc.dram_tensor("dth", [b_core, 1], F32, kind="ExternalInput")[:],
        "cf": nc.dram_tensor("cf", [P, 9], F32, kind="ExternalInput")[:],
    }
    outs = {"out": nc.dram_tensor("out", [b_core, 30], F32, kind="ExternalOutput")[:]}
    with tile.TileContext(nc) as tc:
        emit_core_kernel(nc, tc, ins, outs, n_tiles=n_tiles)
    nc.compile()
    return nc


_NC_CACHE = {}


def _get_nc(n_tiles=NT):
    if n_tiles not in _NC_CACHE:
        _NC_CACHE[n_tiles] = build_nc(n_tiles)
    return _NC_CACHE[n_tiles]


def make_host_inputs(input_angles, forward_params, backward_params, diagonal_params,
                     dt_scale, alpha_real, alpha_imag, beta_real, beta_imag,
                     gamma_real, gamma_imag):
    """Host-side scalar prep shared by kernel() and tests."""
    al = complex(float(alpha_real), float(alpha_imag))
    be = complex(float(beta_real), float(beta_imag))
    ga = complex(float(gamma_real), float(gamma_imag))
    n = np.sqrt(abs(al) ** 2 + abs(be) ** 2 + abs(ga) ** 2 + 1e-9)
    cs = [al / n, be / n, ga / n]
    row = []
    for ck in cs:
        row += [ck.real, ck.imag, -ck.imag]
    cf = np.tile(np.asarray(row, np.float32), (P, 1))
    dth = (0.25 * np.asarray(dt_scale, np.float32)).reshape(-1, 1)
    return cf, dth


def kernel(**inputs):
    from concourse.bass_utils import run_bass_kernel_spmd

    cf, dth = make_host_inputs(**inputs)
    ang = np.ascontiguousarray(np.asarray(inputs["input_angles"], np.float32))
    pf = np.ascontiguousarray(np.asarray(inputs["forward_params"], np.float32))
    pb = np.ascontiguousarray(np.asarray(inputs["backward_params"], np.float32))
    pd = np.ascontiguousarray(np.asarray(inputs["diagonal_params"], np.float32))

    nc = _get_nc(NT)
    in_maps = []
    for c in range(N_CORES):
        r0, r1 = c * B_CORE, (c + 1) * B_CORE
        in_maps.append({
            "input_angles": ang[r0:r1],
            "forward_params": pf[r0:r1],
            "backward_params": pb[r0:r1],
            "diagonal_params": pd[r0:r1],
            "dth": np.ascontiguousarray(dth[r0:r1]),
            "cf": cf,
        })
    res = run_bass_kernel_spmd(nc, in_maps, core_ids=list(range(N_CORES)))
    out = np.concatenate([res.results[c]["out"] for c in range(N_CORES)], axis=0)
    return out.astype(np.float32)



# revision 12
# speedup vs baseline: 1.5219x; 1.5219x over previous
"""Trainium2 Bass kernel for ClassicalReconstructionHydraSSMCore.

Quantum statevector simulation: batch 8192, 10 qubits, three circuits
(forward/backward/diagonal), combine + normalize + Pauli X/Y/Z measure.

Sharding: pure data parallel over batch across 8 cores (1024 each).
Per-core layout: batch on partitions (8 tiles of 128), state on free dim,
fp16 [re(1024) | im(1024)] planes per circuit.

Gate scheme (cost-model driven):
- tensor_scalar products (DVE 2x/4x modes) into compact scratch, then
  packed fp16 TensorTensor adds back into the state (DVE 2x_1p).
- rotation = 11 product ops + 3 adds; CRX = 3 products + 1 add.
- ring1 of layer 0 is "staircased": applied while the initial product
  state is log-doubled, so each CRX costs O(partial state) instead of
  O(full state). Layer-0 1q rotations are folded into the init vectors.
- combine coefficients are folded into the init vectors (linearity), so
  combine is 2 adds.
- ops are spread across DVE / Activation / GpSimd by a greedy balancer
  using the InstructionCostModel rates.
"""

import numpy as np

import concourse.bass as bass
import concourse.tile as tile
from concourse import bacc, mybir

F32 = mybir.dt.float32
F16 = mybir.dt.float16
AOT = mybir.AluOpType
ACTF = mybir.ActivationFunctionType


def _register_axpby():
    """Runtime-register a custom DVE op: out = in0*s0 + in1*s1."""
    import concourse.dve_ops as dve_ops
    from concourse.dve_spec import Spec, Src0, Src1, C0, C1, lower
    from concourse.dve_spec import _has_src1 as has_src1
    from concourse.dve_uop import DveOpSpec

    name = "AXPBY9_ANT"
    for op in dve_ops.OPS:
        if op.name == name:
            return op
    spec = Spec(
        body=Src0 * C0 + Src1 * C1,
        reference=lambda in0, in1, s0, s1, imm2: in0 * s0 + in1 * s1,
    )
    row = dve_ops._CUSTOM_DVE_ROW_BASE + len(dve_ops.OPS)
    assert row < 0x20
    dve_ops._SUB_OPCODE_FOR_NAME[name] = row
    shas = {}
    for ver in ("v3", "v4"):
        s = DveOpSpec(
            name=name, opcode=row, uops=lower(spec, ver=ver), rd1_en=has_src1(spec)
        )
        shas[ver] = s.sha(ver)
    op = dve_ops.DveOp(name, spec, subdim=False, uops_sha=shas)
    dve_ops.OPS.append(op)
    dve_ops.CUSTOM_DVE_SPECS[name] = spec
    return op


AXPBY = _register_axpby()

NQ = 10
DIM = 1 << NQ          # 1024
HD = DIM // 2          # 512
P = 128
N_CORES = 8
B_CORE = 1024
NT = B_CORE // P       # 8 tiles per core
PI_2 = float(np.pi / 2)

FWD, BWD, DIAG = 0, 1, 2


def _ring_gates(c, L):
    """Time-ordered entangler list [(ctrl, tgt, col)] for circuit c, layer L."""
    base = 100 * c + 50 * L
    out = []
    if c in (FWD, DIAG):
        for k in range(NQ):       # ring1: CRX(i, i+1), i ascending
            out.append((k, (k + 1) % NQ, base + 30 + k))
        for k in range(NQ):       # ring2: CRX(i, i-1), i descending
            i = NQ - 1 - k
            out.append((i, (i - 1) % NQ, base + 40 + k))
    else:  # BWD
        for k in range(NQ):       # ring1: CRX(i, i-1), i descending
            i = NQ - 1 - k
            out.append((i, (i - 1) % NQ, base + 30 + k))
        for k in range(NQ):       # ring2: CRX(i, i+1), i ascending
            out.append((k, (k + 1) % NQ, base + 40 + k))
    return out


class Bal:
    """Greedy per-engine load balancer using cost-model rates (ns)."""

    def __init__(self):
        self.load = {"DVE": 0.0, "ACT": 0.0, "POOL": 0.0}

    def _cost(self, eng, kind, elems):
        if eng == "DVE":
            if kind == "ts_p":
                r = 0.26
            elif kind in ("ts_u", "tt_p"):
                r = 0.53
            else:            # tt_u, stt, acc, axpby
                r = 1.05
            return elems * r + 70
        if eng == "ACT":
            return elems * 0.84 + 210
        # POOL
        r = 1.99 if kind.startswith("tt") else 1.39
        return elems * r + 130

    def pick(self, engines, kind, elems):
        best = min(engines, key=lambda e: self.load[e] + self._cost(e, kind, elems))
        self.load[best] += self._cost(best, kind, elems)
        return best

    def add(self, eng, kind, elems):
        self.load[eng] += self._cost(eng, kind, elems)


def emit_core_kernel(nc, tc, ins, outs, n_tiles=NT):
    ang_d = ins["input_angles"]
    par_d = [ins["forward_params"], ins["backward_params"], ins["diagonal_params"]]
    dth_d = ins["dth"]
    cf_d = ins["cf"]
    out_d = outs["out"]

    bal = Bal()

    ttp = nc.gpsimd.tensor_tensor
    ttd = nc.vector.tensor_tensor
    stt_ = lambda out, in0, sc, in1: nc.vector.scalar_tensor_tensor(
        out, in0, sc, in1, op0=AOT.mult, op1=AOT.add
    )

    def TS(out, in_, sc, elems, packed=True, engines=("DVE", "ACT", "POOL")):
        """out = in_ * sc (sc: AP (P,1) or float)."""
        e = bal.pick(engines, "ts_p" if packed else "ts_u", elems)
        if e == "DVE":
            nc.vector.tensor_scalar_mul(out, in_, sc)
        elif e == "ACT":
            nc.scalar.mul(out, in_, sc)
        else:
            nc.gpsimd.tensor_scalar_mul(out, in_, sc)

    def TT(out, a, b, op, elems, packed=True, engines=("DVE", "POOL")):
        e = bal.pick(engines, "tt_p" if packed else "tt_u", elems)
        (ttd if e == "DVE" else ttp)(out, a, b, op=op)

    def ACC(scr, in0, in1, accum, elems):
        """accum = sum(in0 * in1); scr is a same-shape dummy output."""
        bal.add("DVE", "acc", elems)
        nc.vector.scalar_tensor_tensor(
            scr, in0, 0.0, in1, op0=AOT.bypass, op1=AOT.mult, accum_out=accum
        )

    def SQACC(scr, in_, accum, elems):
        """accum = sum(in_^2)."""
        e = bal.pick(("ACT", "DVE"), "acc", elems)
        if e == "ACT":
            nc.scalar.activation(scr, in_, ACTF.Square, accum_out=accum)
        else:
            nc.vector.scalar_tensor_tensor(
                scr, in_, 0.0, in_, op0=AOT.bypass, op1=AOT.mult, accum_out=accum
            )

    def ax(out, x, sx, y, sy, elems):
        bal.add("DVE", "axpby", elems)
        nc.vector._custom_dve(AXPBY, out=out, in0=x, in1=y, s0=sx, s1=sy)

    with (
        tc.tile_pool(name="const", bufs=1) as cpool,
        tc.tile_pool(name="work", bufs=2) as pool,
    ):
        cf_t = cpool.tile([P, 16], F32)
        nc.sync.dma_start(cf_t[:, 0 : cf_d.shape[1]], cf_d[:])
        pi2 = cpool.tile([P, 1], F32)
        nc.gpsimd.memset(pi2[:], PI_2)
        pi2c = pi2[:, 0:1]

        for t in range(n_tiles):
            r0, r1 = t * P, (t + 1) * P
            # ---- loads ----
            par = pool.tile([P, 300], F32, tag="par")
            for c in range(3):
                nc.sync.dma_start(par[:, 100 * c : 100 * (c + 1)], par_d[c][r0:r1, :])
            ang = pool.tile([P, NQ], F32, tag="ang")
            nc.sync.dma_start(ang[:], ang_d[r0:r1, :])
            dth = pool.tile([P, 1], F32, tag="dth")
            nc.sync.dma_start(dth[:], dth_d[r0:r1, :])

            # ---- trig (fp32) ----
            # ScalarE Sin covers [-pi, pi]; quarter angles:
            # u = sin(h/2), w = cos(h/2); sin(h)=2uw, cos(h)=1-2u^2.
            ch = pool.tile([P, 300], F32, tag="ch")
            sh = pool.tile([P, 300], F32, tag="sh")
            nsh = pool.tile([P, 300], F32, tag="nsh")
            trA = pool.tile([P, 100], F32, tag="trA")
            trB = pool.tile([P, 100], F32, tag="trB")

            def emit_trig(dst_s, dst_c, src, scale, scrA, scrB, n):
                bal.add("ACT", "ts_u", 2 * n)
                nc.scalar.activation(dst_s, src, ACTF.Sin, scale=scale)
                nc.scalar.activation(dst_c, src, ACTF.Sin, scale=scale, bias=pi2c)
                TT(scrA, dst_s, dst_c, AOT.mult, n)
                TT(scrB, dst_s, dst_s, AOT.mult, n)
                TS(dst_s, scrA, 2.0, n)
                e = bal.pick(("DVE", "ACT", "POOL"), "ts_u", n)
                if e == "DVE":
                    nc.vector.tensor_scalar(dst_c, scrB, -2.0, 1.0, op0=AOT.mult, op1=AOT.add)
                elif e == "POOL":
                    nc.gpsimd.tensor_scalar(dst_c, scrB, -2.0, 1.0, op0=AOT.mult, op1=AOT.add)
                else:
                    nc.scalar.activation(dst_c, scrB, ACTF.Copy, scale=-2.0, bias=1.0)

            for c in range(3):
                src = par[:, 100 * c : 100 * (c + 1)]
                dst_s = sh[:, 100 * c : 100 * (c + 1)]
                dst_c = ch[:, 100 * c : 100 * (c + 1)]
                if c == DIAG:
                    emit_trig(dst_s, dst_c, src, 0.25, trA[:], trB[:], 100)
                else:
                    emit_trig(dst_s, dst_c, src, dth[:, 0:1], trA[:], trB[:], 100)
                    # fix CRX cols (30-49, 80-99): no dt factor
                    lx = lambda ap: ap.rearrange("p (l x) -> p l x", l=2, x=50)[:, :, 30:50]
                    emit_trig(
                        lx(dst_s), lx(dst_c), lx(src), 0.25,
                        trA[:, 0:40].rearrange("p (l x) -> p l x", l=2, x=20),
                        trB[:, 0:40].rearrange("p (l x) -> p l x", l=2, x=20),
                        40,
                    )
            TS(nsh[:], sh[:], -1.0, 300)

            angc = pool.tile([P, NQ], F32, tag="angc")
            angs = pool.tile([P, NQ], F32, tag="angs")
            emit_trig(angs[:], angc[:], ang[:], 0.25, trA[:, 0:NQ], trB[:, 0:NQ], NQ)
            a3c = pool.tile([P, 30], F32, tag="a3c")
            a3s = pool.tile([P, 30], F32, tag="a3s")
            for dst, srcv in ((a3c, angc), (a3s, angs)):
                TS(dst[:, 0:10], srcv[:], 1.0, 10, packed=False)
                TS(dst[:, 10:20], srcv[:, ::-1], 1.0, 10, packed=False)
                TS(dst[:, 20:30], srcv[:], 1.0, 10, packed=False)

            # ---- u-coefficients per layer (fp32, (128,30)) ----
            ch3 = ch[:].rearrange("p (c x) -> p c x", c=3, x=100)
            sh3 = sh[:].rearrange("p (c x) -> p c x", c=3, x=100)
            m1 = pool.tile([P, 30], F32, tag="m1")
            m2 = pool.tile([P, 30], F32, tag="m2")
            m3 = pool.tile([P, 30], F32, tag="m3")
            m4 = pool.tile([P, 30], F32, tag="m4")
            w1 = pool.tile([P, 30], F32, tag="w1")
            w2 = pool.tile([P, 30], F32, tag="w2")
            V = lambda tl: tl[:].rearrange("p (c g) -> p c g", c=3, g=10)
            U = []
            for L in range(2):
                ca = ch3[:, :, 50 * L : 50 * L + 30 : 3]
                cb = ch3[:, :, 50 * L + 1 : 50 * L + 30 : 3]
                cg = ch3[:, :, 50 * L + 2 : 50 * L + 30 : 3]
                sa = sh3[:, :, 50 * L : 50 * L + 30 : 3]
                sb = sh3[:, :, 50 * L + 1 : 50 * L + 30 : 3]
                sg = sh3[:, :, 50 * L + 2 : 50 * L + 30 : 3]
                u = {
                    k: pool.tile([P, 30], F32, tag=f"u{k}{L}", name=f"u{k}{L}")
                    for k in ("p", "q", "nq", "r", "nr", "s", "ns")
                }
                TT(V(m1), cb, ca, AOT.mult, 30)
                TT(V(m2), sb, sa, AOT.mult, 30)
                TT(V(m3), sb, ca, AOT.mult, 30)
                TT(V(m4), cb, sa, AOT.mult, 30)
                TT(V(w1), cg, V(m1), AOT.mult, 30)
                TT(V(w2), sg, V(m2), AOT.mult, 30)
                TT(V(u["p"]), V(w1), V(w2), AOT.add, 30)
                TT(V(w1), cg, V(m2), AOT.mult, 30)
                TT(V(w2), sg, V(m1), AOT.mult, 30)
                TT(V(u["q"]), V(w1), V(w2), AOT.subtract, 30)
                TT(V(w1), cg, V(m3), AOT.mult, 30)
                TT(V(w2), sg, V(m4), AOT.mult, 30)
                TT(V(u["nr"]), V(w1), V(w2), AOT.add, 30)
                TT(V(w1), sg, V(m3), AOT.mult, 30)
                TT(V(w2), cg, V(m4), AOT.mult, 30)
                TT(V(u["s"]), V(w1), V(w2), AOT.subtract, 30)
                TS(u["nq"][:], u["q"][:], -1.0, 30)
                TS(u["r"][:], u["nr"][:], -1.0, 30)
                TS(u["ns"][:], u["s"][:], -1.0, 30)
                U.append(u)

            # ---- v vectors: layer-0 rotations folded into init ----
            u0 = U[0]
            v0r = pool.tile([P, 30], F32, tag="v0r")
            v0i = pool.tile([P, 30], F32, tag="v0i")
            v1r = pool.tile([P, 30], F32, tag="v1r")
            v1i = pool.tile([P, 30], F32, tag="v1i")
            nv0i = pool.tile([P, 30], F32, tag="nv0i")
            nv1i = pool.tile([P, 30], F32, tag="nv1i")
            for dst, t1, t2 in (
                (v0r, ("p", a3c), ("r", a3s)),
                (v0i, ("q", a3c), ("s", a3s)),
                (v1r, ("nr", a3c), ("p", a3s)),
                (v1i, ("s", a3c), ("nq", a3s)),
            ):
                TT(w1[:], u0[t1[0]][:], t1[1][:], AOT.mult, 30)
                TT(w2[:], u0[t2[0]][:], t2[1][:], AOT.mult, 30)
                TT(dst[:], w1[:], w2[:], AOT.add, 30)

            # fold combine coefficient c_k into wire col (10c) of each circuit's v
            # (linearity: c*psi = circuit applied to c*psi0). cf cols per c:
            # [cr, ci, nci].
            for c in range(3):
                j = 10 * c
                crc = cf_t[:, 3 * c : 3 * c + 1]
                cic = cf_t[:, 3 * c + 1 : 3 * c + 2]
                ncic = cf_t[:, 3 * c + 2 : 3 * c + 3]
                for vr, vi in ((v0r, v0i), (v1r, v1i)):
                    ax(w1[:, 0:1], vr[:, j : j + 1], crc, vi[:, j : j + 1], ncic, 1)
                    ax(w2[:, 0:1], vi[:, j : j + 1], crc, vr[:, j : j + 1], cic, 1)
                    TS(vr[:, j : j + 1], w1[:, 0:1], 1.0, 1, packed=False, engines=("DVE",))
                    TS(vi[:, j : j + 1], w2[:, 0:1], 1.0, 1, packed=False, engines=("DVE",))
            TS(nv0i[:], v0i[:], -1.0, 30)
            TS(nv1i[:], v1i[:], -1.0, 30)

            # ---- staircase w vectors: w = RX(theta_ring1[j-1]) v(g=j) ----
            # (p, 3, 9) grids: circuit-major columns.
            wt = {k: pool.tile([P, 27], F32, tag=f"wt{k}", name=f"wt{k}") for k in
                  ("w0r", "w0i", "w1r", "w1i", "nw0i", "nw1i")}
            W9 = lambda tl: tl[:].rearrange("p (c g) -> p c g", c=3, g=9)
            csv = ch3[:, :, 30:39]
            snv = sh3[:, :, 30:39]
            vv = lambda tl: V(tl)[:, :, 1:10]
            wm1 = pool.tile([P, 27], F32, tag="wm1")
            wm2 = pool.tile([P, 27], F32, tag="wm2")
            for dst, (f1a, f1b), (f2a, f2b), op in (
                ("w0r", (csv, v0r), (snv, v1i), AOT.add),      # c*v0r + sn*v1i
                ("w0i", (csv, v0i), (snv, v1r), AOT.subtract), # c*v0i - sn*v1r
                ("w1r", (snv, v0i), (csv, v1r), AOT.add),      # sn*v0i + c*v1r
                ("w1i", (csv, v1i), (snv, v0r), AOT.subtract), # c*v1i - sn*v0r
            ):
                TT(W9(wm1), f1a, vv(f1b), AOT.mult, 27)
                TT(W9(wm2), f2a, vv(f2b), AOT.mult, 27)
                TT(W9(wt[dst]), W9(wm1), W9(wm2), op, 27)
            TS(wt["nw0i"][:], wt["w0i"][:], -1.0, 27)
            TS(wt["nw1i"][:], wt["w1i"][:], -1.0, 27)

            # ---- state build: staircase (init + ring1-L0) ----
            st = [pool.tile([P, 2 * DIM], F16, tag=f"st{c}", name=f"st{c}") for c in range(3)]
            PR = [pool.tile([P, 8192], F16, tag=f"PR{c}", name=f"PR{c}") for c in range(3)]
            ES = [PR[c][:, 0:2048] for c in range(3)]

            col = lambda tl, j: tl[:, j : j + 1]
            wcol = lambda k, c, g: wt[k][:, 9 * c + g : 9 * c + g + 1]

            # E_1 init: [v0(g=0), v1(g=0)] (fp32 -> fp16 convert copies)
            for c in range(3):
                j0 = 10 * c
                TS(ES[c][:, 0:1], col(v0r, j0), 1.0, 1, packed=False, engines=("DVE",))
                TS(ES[c][:, 1:2], col(v1r, j0), 1.0, 1, packed=False, engines=("DVE",))
                TS(ES[c][:, 1024:1025], col(v0i, j0), 1.0, 1, packed=False, engines=("DVE",))
                TS(ES[c][:, 1025:1026], col(v1i, j0), 1.0, 1, packed=False, engines=("DVE",))

            # staircase steps j=1..9: extend with wire g=j, gate CRX(ring1[j-1])
            # FWD/DIAG: new dim innermost; BWD: new dim outermost.
            # buffers: E_j in ES for odd j... E_j lives in ES if j odd else st.
            for j in range(1, NQ):
                W = 1 << j  # current complex width
                for c in range(3):
                    src_b = ES[c] if (j % 2 == 1) else st[c]
                    dst_b = st[c] if (j % 2 == 1) else ES[c]
                    jc = 10 * c + j
                    inner = c in (FWD, DIAG)
                    for cb in (0, 1):
                        if cb == 0:
                            ur = [col(v0r, jc), col(v1r, jc)]
                            ui = [col(v0i, jc), col(v1i, jc)]
                            nui = [col(nv0i, jc), col(nv1i, jc)]
                        else:
                            ur = [wcol("w0r", c, j - 1), wcol("w1r", c, j - 1)]
                            ui = [wcol("w0i", c, j - 1), wcol("w1i", c, j - 1)]
                            nui = [wcol("nw0i", c, j - 1), wcol("nw1i", c, j - 1)]
                        for tt in (0, 1):
                            for pl in (0, 1):  # 0: re-out, 1: im-out
                                po = 1024 * pl
                                if inner:
                                    sv = lambda plane: (
                                        src_b[:, 1024 * plane : 1024 * plane + W]
                                        .rearrange("p (x c2) -> p x c2", x=W // 2, c2=2)[:, :, cb]
                                    )
                                    dv = (
                                        dst_b[:, po : po + 2 * W]
                                        .rearrange("p (x c2 t2) -> p x c2 t2", x=W // 2, c2=2, t2=2)
                                        [:, :, cb, tt]
                                    )
                                else:
                                    sv = lambda plane: src_b[
                                        :, 1024 * plane + cb * (W // 2) : 1024 * plane + (cb + 1) * (W // 2)
                                    ]
                                    dv = dst_b[
                                        :, po + tt * W + cb * (W // 2) : po + tt * W + (cb + 1) * (W // 2)
                                    ]
                                if pl == 0:
                                    ax(dv, sv(0), ur[tt], sv(1), nui[tt], W // 2)
                                else:
                                    ax(dv, sv(1), ur[tt], sv(0), ui[tt], W // 2)
            # E_10 lands in st (j=9 odd -> dst st) for all circuits.

            # ---- full-state gate emitters ----
            def emit_rot(c, w, u, j):
                """SU(2) rotation on wire w; coeffs u[...][:, j]."""
                sp = col(u["p"], j); sq = col(u["q"], j); snq = col(u["nq"], j)
                sr = col(u["r"], j); snr = col(u["nr"], j)
                ss = col(u["s"], j); sns = col(u["ns"], j)
                stc = st[c]
                pr = PR[c]
                if w <= 8:
                    inner = 1 << (9 - w)
                    m = 1 << (w + 1)
                    fv = stc[:].rearrange("p (m t i) -> p m t i", m=m, t=2, i=inner)
                    gv = stc[:].rearrange(
                        "p (pl o t i) -> p pl o t i", pl=2, o=m // 2, t=2, i=inner
                    )
                    PA = pr[:, 0:2048].rearrange("p (m t i) -> p m t i", m=m, t=2, i=inner)
                    PB = pr[:, 2048:4096].rearrange("p (m t i) -> p m t i", m=m, t=2, i=inner)
                    pqv = lambda base: pr[:, base : base + 2048].rearrange(
                        "p (pl o t i) -> p pl o t i", pl=2, o=m // 2, t=2, i=inner
                    )
                    PC = pqv(4096)
                    PD = pqv(6144)
                    pk = inner >= 2
                    TS(PA, fv, sp, 2048, packed=pk)
                    TS(PB[:, :, 0, :], fv[:, :, 1, :], sr, 1024, packed=pk)
                    TS(PB[:, :, 1, :], fv[:, :, 0, :], snr, 1024, packed=pk)
                    TS(PC[:, 0, :, 0, :], gv[:, 1, :, 0, :], snq, 512, packed=pk)
                    TS(PC[:, 1, :, 0, :], gv[:, 0, :, 0, :], sq, 512, packed=pk)
                    TS(PC[:, 0, :, 1, :], gv[:, 1, :, 1, :], sq, 512, packed=pk)
                    TS(PC[:, 1, :, 1, :], gv[:, 0, :, 1, :], snq, 512, packed=pk)
                    TS(PD[:, 0, :, 0, :], gv[:, 1, :, 1, :], sns, 512, packed=pk)
                    TS(PD[:, 1, :, 0, :], gv[:, 0, :, 1, :], ss, 512, packed=pk)
                    TS(PD[:, 0, :, 1, :], gv[:, 1, :, 0, :], sns, 512, packed=pk)
                    TS(PD[:, 1, :, 1, :], gv[:, 0, :, 0, :], ss, 512, packed=pk)
                    TT(pr[:, 0:2048], pr[:, 0:2048], pr[:, 2048:4096], AOT.add, 2048)
                    TT(pr[:, 4096:6144], pr[:, 4096:6144], pr[:, 6144:8192], AOT.add, 2048)
                    PCm = pr[:, 4096:6144].rearrange(
                        "p (m t i) -> p m t i", m=m, t=2, i=inner
                    )
                    TT(fv, PA, PCm, AOT.add, 2048, packed=pk)
                else:  # w == 9: pairs are adjacent elements
                    fv = stc[:].rearrange("p (m t) -> p m t", m=1024, t=2)
                    gv = stc[:].rearrange("p (pl o t) -> p pl o t", pl=2, o=512, t=2)
                    PA = pr[:, 0:2048].rearrange("p (m t) -> p m t", m=1024, t=2)
                    PB = pr[:, 2048:4096].rearrange("p (m t) -> p m t", m=1024, t=2)
                    pqv = lambda base: pr[:, base : base + 2048].rearrange(
                        "p (pl o t) -> p pl o t", pl=2, o=512, t=2
                    )
                    PC = pqv(4096)
                    PD = pqv(6144)
                    TS(pr[:, 0:2048], stc[:], sp, 2048)
                    TS(PB[:, :, 0], fv[:, :, 1], sr, 1024, packed=False)
                    TS(PB[:, :, 1], fv[:, :, 0], snr, 1024, packed=False)
                    TS(PC[:, 0, :, 0], gv[:, 1, :, 0], snq, 512, packed=False)
                    TS(PC[:, 1, :, 0], gv[:, 0, :, 0], sq, 512, packed=False)
                    TS(PC[:, 0, :, 1], gv[:, 1, :, 1], sq, 512, packed=False)
                    TS(PC[:, 1, :, 1], gv[:, 0, :, 1], snq, 512, packed=False)
                    TS(PD[:, 0, :, 0], gv[:, 1, :, 1], sns, 512, packed=False)
                    TS(PD[:, 1, :, 0], gv[:, 0, :, 1], ss, 512, packed=False)
                    TS(PD[:, 0, :, 1], gv[:, 1, :, 0], sns, 512, packed=False)
                    TS(PD[:, 1, :, 1], gv[:, 0, :, 0], ss, 512, packed=False)
                    TT(pr[:, 0:2048], pr[:, 0:2048], pr[:, 2048:4096], AOT.add, 2048)
                    TT(pr[:, 4096:6144], pr[:, 4096:6144], pr[:, 6144:8192], AOT.add, 2048)
                    TT(stc[:], pr[:, 0:2048], pr[:, 4096:6144], AOT.add, 2048)

            def emit_crx(c, ctrl, tgt, cl):
                cc = col(ch, cl)
                ssc = col(sh, cl)
                nsc = col(nsh, cl)
                stc = st[c]
                pr = PR[c]
                hi, lo = min(ctrl, tgt), max(ctrl, tgt)
                PQ = pr[:, 0:1024]
                PS = pr[:, 1024:2048]
                if lo - hi == 1:
                    a = 1 << hi
                    z = 1 << (8 - hi)
                    v6 = stc[:].rearrange(
                        "p (pl a x y z) -> p pl a x y z", pl=2, a=a, x=2, y=2, z=z
                    )
                    vm = stc[:].rearrange(
                        "p (pla x y z) -> p pla x y z", pla=2 * a, x=2, y=2, z=z
                    )
                    PSm = PS.rearrange("p (pla y z) -> p pla y z", pla=2 * a, y=2, z=z)
                    if ctrl < tgt:
                        # ctrl bit = x, tgt bit = y
                        Q = vm[:, :, 1, :, :]                     # (p, pla, y, z)
                        Qp = lambda pl, y: v6[:, pl, :, 1, y, :]  # (p, a, z)
                        QPv = PQ.rearrange("p (pla y z) -> p pla y z", pla=2 * a, y=2, z=z)
                        PSv = PS.rearrange("p (pl a y z) -> p pl a y z", pl=2, a=a, y=2, z=z)
                        TS(QPv, Q, cc, 1024)
                        # s-products, tgt-swapped, sign per plane
                        TS(PSv[:, 0, :, 0, :], Qp(1, 1), ssc, 256)
                        TS(PSv[:, 0, :, 1, :], Qp(1, 0), ssc, 256)
                        TS(PSv[:, 1, :, 0, :], Qp(0, 1), nsc, 256)
                        TS(PSv[:, 1, :, 1, :], Qp(0, 0), nsc, 256)
                        TT(Q, QPv, PSm, AOT.add, 1024)
                    else:
                        # tgt bit = x, ctrl bit = y
                        Q = vm[:, :, :, 1, :]                     # (p, pla, x, z)
                        Qp = lambda pl, x: v6[:, pl, :, x, 1, :]
                        QPv = PQ.rearrange("p (pla x z) -> p pla x z", pla=2 * a, x=2, z=z)
                        PSv = PS.rearrange("p (pl a x z) -> p pl a x z", pl=2, a=a, x=2, z=z)
                        TS(QPv, Q, cc, 1024)
                        TS(PSv[:, 0, :, 0, :], Qp(1, 1), ssc, 256)
                        TS(PSv[:, 0, :, 1, :], Qp(1, 0), ssc, 256)
                        TS(PSv[:, 1, :, 0, :], Qp(0, 1), nsc, 256)
                        TS(PSv[:, 1, :, 1, :], Qp(0, 0), nsc, 256)
                        TT(Q, QPv, PSm, AOT.add, 1024)
                else:
                    raise AssertionError("wrap pair handled by emit_crx_wrap")

            def emit_crx_wrap(c, ctrl, tgt, cl):
                cc = col(ch, cl)
                ssc = col(sh, cl)
                nsc = col(nsh, cl)
                stc = st[c]
                pr = PR[c]
                PQ = pr[:, 0:1024]
                PS = pr[:, 1024:2048]
                v5 = stc[:].rearrange(
                    "p (pl x mid y) -> p pl x mid y", pl=2, x=2, mid=256, y=2
                )
                if ctrl == 0:
                    # ctrl = x (stride 512), tgt = y (stride 1): quarter x=1
                    Q = v5[:, :, 1, :, :]                      # (p, pl, mid, y)
                    Qp = lambda pl, y: v5[:, pl, 1, :, y]      # (p, mid) stride 2
                    QPv = PQ.rearrange("p (pl mid y) -> p pl mid y", pl=2, mid=256, y=2)
                    PSv = PS.rearrange("p (pl mid y) -> p pl mid y", pl=2, mid=256, y=2)
                    TS(QPv, Q, cc, 1024)
                    TS(PSv[:, 0, :, 0], Qp(1, 1), ssc, 256, packed=False)
                    TS(PSv[:, 0, :, 1], Qp(1, 0), ssc, 256, packed=False)
                    TS(PSv[:, 1, :, 0], Qp(0, 1), nsc, 256, packed=False)
                    TS(PSv[:, 1, :, 1], Qp(0, 0), nsc, 256, packed=False)
                    TT(Q, QPv, PSv, AOT.add, 1024)
                else:
                    # ctrl = 9 (y, stride 1), tgt = 0 (x, stride 512): quarter y=1
                    Q = v5[:, :, :, :, 1]                      # (p, pl, x, mid)
                    Qp = lambda pl, x: v5[:, pl, x, :, 1]      # (p, mid) stride 2
                    QPv = PQ.rearrange("p (pl x mid) -> p pl x mid", pl=2, x=2, mid=256)
                    PSv = PS.rearrange("p (pl x mid) -> p pl x mid", pl=2, x=2, mid=256)
                    TS(QPv, Q, cc, 1024, packed=False)
                    TS(PSv[:, 0, 0, :], Qp(1, 1), ssc, 256, packed=False)
                    TS(PSv[:, 0, 1, :], Qp(1, 0), ssc, 256, packed=False)
                    TS(PSv[:, 1, 0, :], Qp(0, 1), nsc, 256, packed=False)
                    TS(PSv[:, 1, 1, :], Qp(0, 0), nsc, 256, packed=False)
                    TT(Q, QPv, PSv, AOT.add, 1024, packed=False)

            def emit_crx_any(c, ctrl, tgt, cl):
                if abs(ctrl - tgt) == 1:
                    emit_crx(c, ctrl, tgt, cl)
                else:
                    emit_crx_wrap(c, ctrl, tgt, cl)

            # ---- gate sequences (interleave circuits for engine overlap) ----
            seqs = []
            for c in range(3):
                g = []
                ring0 = _ring_gates(c, 0)
                g.append(("crx", ring0[9]))            # ring1-L0 wrap gate
                for e in ring0[10:20]:
                    g.append(("crx", e))               # ring2-L0
                for gg in range(NQ):                   # rotations L1 (commuting)
                    w = gg if c != BWD else 9 - gg
                    g.append(("rot", (w, 10 * c + gg)))
                for e in _ring_gates(c, 1):
                    g.append(("crx", e))               # ring1-L1 + ring2-L1
                seqs.append(g)
            for step in range(len(seqs[0])):
                for c in range(3):
                    kind, arg = seqs[c][step]
                    if kind == "crx":
                        ctrl, tgt, cl = arg
                        emit_crx_any(c, ctrl, tgt, cl)
                    else:
                        w, j = arg
                        emit_rot(c, w, U[1], j)

            # ---- combine: st0 += st1 + st2 (coeffs already folded) ----
            TT(st[0][:], st[0][:], st[1][:], AOT.add, 2048)
            TT(st[0][:], st[0][:], st[2][:], AOT.add, 2048)

            # ---- measure ----
            acc = st[0]
            Rt = PR[0][:, 0:2048]      # R = -i * acc
            msS = PR[1][:, 0:2048]     # dummy accum outputs
            ms1 = PR[1][:, 2048:3072]
            crX = pool.tile([P, NQ], F32, tag="crX")
            crY = pool.tile([P, NQ], F32, tag="crY")
            hZ = pool.tile([P, NQ], F32, tag="hZ")
            scol = pool.tile([P, 8], F32, tag="scol")
            TS(Rt[:, 0:1024], acc[:, 1024:2048], 1.0, 1024)
            TS(Rt[:, 1024:2048], acc[:, 0:1024], -1.0, 1024)

            SQACC(msS, acc[:], scol[:, 0:1], 2048)
            for w in range(NQ):
                if w <= 8:
                    inner = 1 << (9 - w)
                    m = 1 << (w + 1)
                    fv = acc[:].rearrange("p (m t i) -> p m t i", m=m, t=2, i=inner)
                    rv = Rt.rearrange("p (m t i) -> p m t i", m=m, t=2, i=inner)
                    a0 = fv[:, :, 0, :]
                    a1 = fv[:, :, 1, :]
                    rr1 = rv[:, :, 1, :]
                    ms = ms1.rearrange("p (m i) -> p m i", m=m, i=inner)
                else:
                    fv = acc[:].rearrange("p (m t) -> p m t", m=1024, t=2)
                    rv = Rt.rearrange("p (m t) -> p m t", m=1024, t=2)
                    a0 = fv[:, :, 0]
                    a1 = fv[:, :, 1]
                    rr1 = rv[:, :, 1]
                    ms = ms1
                ACC(ms, a0, a1, crX[:, w : w + 1], 1024)
                ACC(ms, a0, rr1, crY[:, w : w + 1], 1024)
                SQACC(ms, a1, hZ[:, w : w + 1], 1024)

            # inv = 1/(S + 1e-9); s1 = 2*inv; s2 = -2*inv; sz = S*inv
            nc.vector.tensor_scalar(scol[:, 1:2], scol[:, 0:1], 1e-9, None, op0=AOT.add)
            nc.vector.reciprocal(scol[:, 2:3], scol[:, 1:2])
            nc.vector.tensor_scalar(scol[:, 3:4], scol[:, 2:3], 2.0, None, op0=AOT.mult)
            nc.vector.tensor_scalar(scol[:, 4:5], scol[:, 2:3], -2.0, None, op0=AOT.mult)
            ttd(scol[:, 5:6], scol[:, 0:1], scol[:, 2:3], op=AOT.mult)

            out30 = pool.tile([P, 30], F32, tag="out30")
            nc.vector.tensor_scalar_mul(out30[:, 0:10], crX[:], scol[:, 3:4])
            nc.vector.tensor_scalar_mul(out30[:, 10:20], crY[:], scol[:, 3:4])
            szb = scol[:, 5:6].broadcast_to([P, 1, NQ])
            nc.vector.scalar_tensor_tensor(
                out30[:, 20:30].unsqueeze(1), hZ[:].unsqueeze(1), scol[:, 4:5], szb,
                op0=AOT.mult, op1=AOT.add,
            )
            nc.sync.dma_start(out_d[r0:r1, :], out30[:])


def build_nc(n_tiles=NT, b_core=None):
    if b_core is None:
        b_core = n_tiles * P
    nc = bacc.Bacc("TRN2", target_bir_lowering=False)
    ins = {
        "input_angles": nc.dram_tensor("input_angles", [b_core, NQ], F32, kind="ExternalInput")[:],
        "forward_params": nc.dram_tensor("forward_params", [b_core, 100], F32, kind="ExternalInput")[:],
        "backward_params": nc.dram_tensor("backward_params", [b_core, 100], F32, kind="ExternalInput")[:],
        "diagonal_params": nc.dram_tensor("diagonal_params", [b_core, 100], F32, kind="ExternalInput")[:],
        "dth": nc.dram_tensor("dth", [b_core, 1], F32, kind="ExternalInput")[:],
        "cf": nc.dram_tensor("cf", [P, 9], F32, kind="ExternalInput")[:],
    }
    outs = {"out": nc.dram_tensor("out", [b_core, 30], F32, kind="ExternalOutput")[:]}
    with tile.TileContext(nc) as tc:
        emit_core_kernel(nc, tc, ins, outs, n_tiles=n_tiles)
    nc.compile()
    return nc


_NC_CACHE = {}


def _get_nc(n_tiles=NT):
    if n_tiles not in _NC_CACHE:
        _NC_CACHE[n_tiles] = build_nc(n_tiles)
    return _NC_CACHE[n_tiles]


def make_host_inputs(input_angles, forward_params, backward_params, diagonal_params,
                     dt_scale, alpha_real, alpha_imag, beta_real, beta_imag,
                     gamma_real, gamma_imag):
    """Host-side scalar prep shared by kernel() and tests."""
    al = complex(float(alpha_real), float(alpha_imag))
    be = complex(float(beta_real), float(beta_imag))
    ga = complex(float(gamma_real), float(gamma_imag))
    n = np.sqrt(abs(al) ** 2 + abs(be) ** 2 + abs(ga) ** 2 + 1e-9)
    cs = [al / n, be / n, ga / n]
    row = []
    for ck in cs:
        row += [ck.real, ck.imag, -ck.imag]
    cf = np.tile(np.asarray(row, np.float32), (P, 1))
    dth = (0.25 * np.asarray(dt_scale, np.float32)).reshape(-1, 1)
    return cf, dth


def kernel(**inputs):
    from concourse.bass_utils import run_bass_kernel_spmd

    cf, dth = make_host_inputs(**inputs)
    ang = np.ascontiguousarray(np.asarray(inputs["input_angles"], np.float32))
    pf = np.ascontiguousarray(np.asarray(inputs["forward_params"], np.float32))
    pb = np.ascontiguousarray(np.asarray(inputs["backward_params"], np.float32))
    pd = np.ascontiguousarray(np.asarray(inputs["diagonal_params"], np.float32))

    nc = _get_nc(NT)
    in_maps = []
    for c in range(N_CORES):
        r0, r1 = c * B_CORE, (c + 1) * B_CORE
        in_maps.append({
            "input_angles": ang[r0:r1],
            "forward_params": pf[r0:r1],
            "backward_params": pb[r0:r1],
            "diagonal_params": pd[r0:r1],
            "dth": np.ascontiguousarray(dth[r0:r1]),
            "cf": cf,
        })
    res = run_bass_kernel_spmd(nc, in_maps, core_ids=list(range(N_CORES)))
    out = np.concatenate([res.results[c]["out"] for c in range(N_CORES)], axis=0)
    return out.astype(np.float32)


# revision 18
# speedup vs baseline: 1.6759x; 1.1012x over previous
"""Trainium2 Bass kernel for ClassicalReconstructionHydraSSMCore.

Quantum statevector simulation: batch 8192, 10 qubits, three circuits
(forward/backward/diagonal), combine + normalize + Pauli X/Y/Z measure.

Sharding: pure data parallel over batch across 8 cores (1024 each).
Per-core layout: batch on partitions (8 tiles of 128), state on free dim,
fp16 [re(1024) | im(1024)] planes per circuit.

Gate scheme (cost-model driven):
- tensor_scalar products (DVE 2x/4x modes) into compact scratch, then
  packed fp16 TensorTensor adds back into the state (DVE 2x_1p).
- rotation = 11 product ops + 3 adds; CRX = 3 products + 1 add.
- ring1 of layer 0 is "staircased": applied while the initial product
  state is log-doubled, so each CRX costs O(partial state) instead of
  O(full state). Layer-0 1q rotations are folded into the init vectors.
- combine coefficients are folded into the init vectors (linearity), so
  combine is 2 adds.
- ops are spread across DVE / Activation / GpSimd by a greedy balancer
  using the InstructionCostModel rates.
"""

import numpy as np

import concourse.bass as bass
import concourse.tile as tile
from concourse import bacc, mybir

F32 = mybir.dt.float32
F16 = mybir.dt.float16
AOT = mybir.AluOpType
ACTF = mybir.ActivationFunctionType


def _register_axpby():
    """Runtime-register a custom DVE op: out = in0*s0 + in1*s1."""
    import concourse.dve_ops as dve_ops
    from concourse.dve_spec import Spec, Src0, Src1, C0, C1, lower
    from concourse.dve_spec import _has_src1 as has_src1
    from concourse.dve_uop import DveOpSpec

    name = "AXPBY9_ANT"
    for op in dve_ops.OPS:
        if op.name == name:
            return op
    spec = Spec(
        body=Src0 * C0 + Src1 * C1,
        reference=lambda in0, in1, s0, s1, imm2: in0 * s0 + in1 * s1,
    )
    row = dve_ops._CUSTOM_DVE_ROW_BASE + len(dve_ops.OPS)
    assert row < 0x20
    dve_ops._SUB_OPCODE_FOR_NAME[name] = row
    shas = {}
    for ver in ("v3", "v4"):
        s = DveOpSpec(
            name=name, opcode=row, uops=lower(spec, ver=ver), rd1_en=has_src1(spec)
        )
        shas[ver] = s.sha(ver)
    op = dve_ops.DveOp(name, spec, subdim=False, uops_sha=shas)
    dve_ops.OPS.append(op)
    dve_ops.CUSTOM_DVE_SPECS[name] = spec
    return op


AXPBY = _register_axpby()

NQ = 10
DIM = 1 << NQ          # 1024
HD = DIM // 2          # 512
P = 128
N_CORES = 8
B_CORE = 1024
NT = B_CORE // P       # 8 tiles per core
PI_2 = float(np.pi / 2)

FWD, BWD, DIAG = 0, 1, 2


def _ring_gates(c, L):
    """Time-ordered entangler list [(ctrl, tgt, col)] for circuit c, layer L."""
    base = 100 * c + 50 * L
    out = []
    if c in (FWD, DIAG):
        for k in range(NQ):       # ring1: CRX(i, i+1), i ascending
            out.append((k, (k + 1) % NQ, base + 30 + k))
        for k in range(NQ):       # ring2: CRX(i, i-1), i descending
            i = NQ - 1 - k
            out.append((i, (i - 1) % NQ, base + 40 + k))
    else:  # BWD
        for k in range(NQ):       # ring1: CRX(i, i-1), i descending
            i = NQ - 1 - k
            out.append((i, (i - 1) % NQ, base + 30 + k))
        for k in range(NQ):       # ring2: CRX(i, i+1), i ascending
            out.append((k, (k + 1) % NQ, base + 40 + k))
    return out


class Bal:
    """Greedy per-engine load balancer using cost-model rates (ns)."""

    def __init__(self):
        self.load = {"DVE": 0.0, "ACT": 0.0, "POOL": 0.0}

    def _cost(self, eng, kind, elems):
        if eng == "DVE":
            if kind == "ts_p":
                r = 0.26
            elif kind in ("ts_u", "tt_p"):
                r = 0.53
            else:            # tt_u, stt, acc, axpby
                r = 1.05
            return elems * r + 70
        if eng == "ACT":
            return elems * 0.84 + 210
        # POOL
        r = 1.99 if kind.startswith("tt") else 1.39
        return elems * r + 130

    def pick(self, engines, kind, elems):
        best = min(engines, key=lambda e: self.load[e] + self._cost(e, kind, elems))
        self.load[best] += self._cost(best, kind, elems)
        return best

    def add(self, eng, kind, elems):
        self.load[eng] += self._cost(eng, kind, elems)


def emit_core_kernel(nc, tc, ins, outs, n_tiles=NT):
    ang_d = ins["input_angles"]
    par_d = [ins["forward_params"], ins["backward_params"], ins["diagonal_params"]]
    dth_d = ins["dth"]
    cf_d = ins["cf"]
    out_d = outs["out"]

    bal = Bal()

    ttp = nc.gpsimd.tensor_tensor
    ttd = nc.vector.tensor_tensor
    stt_ = lambda out, in0, sc, in1: nc.vector.scalar_tensor_tensor(
        out, in0, sc, in1, op0=AOT.mult, op1=AOT.add
    )

    def TS(out, in_, sc, elems, packed=True, engines=("DVE", "ACT", "POOL")):
        """out = in_ * sc (sc: AP (P,1) or float)."""
        e = bal.pick(engines, "ts_p" if packed else "ts_u", elems)
        if e == "DVE":
            nc.vector.tensor_scalar_mul(out, in_, sc)
        elif e == "ACT":
            nc.scalar.mul(out, in_, sc)
        else:
            nc.gpsimd.tensor_scalar_mul(out, in_, sc)

    def TT(out, a, b, op, elems, packed=True, engines=("DVE", "POOL")):
        e = bal.pick(engines, "tt_p" if packed else "tt_u", elems)
        (ttd if e == "DVE" else ttp)(out, a, b, op=op)

    def ACC(scr, in0, in1, accum, elems):
        """accum = sum(in0 * in1); scr is a same-shape dummy output."""
        bal.add("DVE", "acc", elems)
        nc.vector.scalar_tensor_tensor(
            scr, in0, 0.0, in1, op0=AOT.bypass, op1=AOT.mult, accum_out=accum
        )

    def SQACC(scr, in_, accum, elems):
        """accum = sum(in_^2)."""
        e = bal.pick(("ACT", "DVE"), "acc", elems)
        if e == "ACT":
            nc.scalar.activation(scr, in_, ACTF.Square, accum_out=accum)
        else:
            nc.vector.scalar_tensor_tensor(
                scr, in_, 0.0, in_, op0=AOT.bypass, op1=AOT.mult, accum_out=accum
            )

    def ax(out, x, sx, y, sy, elems):
        bal.add("DVE", "axpby", elems)
        nc.vector._custom_dve(AXPBY, out=out, in0=x, in1=y, s0=sx, s1=sy)

    with (
        tc.tile_pool(name="const", bufs=1) as cpool,
        tc.tile_pool(name="work", bufs=2) as bigpool,
        tc.tile_pool(name="coef", bufs=3) as pool,
    ):
        cf_t = cpool.tile([P, 16], F32)
        nc.sync.dma_start(cf_t[:, 0 : cf_d.shape[1]], cf_d[:])
        pi2 = cpool.tile([P, 1], F32)
        nc.gpsimd.memset(pi2[:], PI_2)
        pi2c = pi2[:, 0:1]

        col = lambda tl, j: tl[:, j : j + 1]

        def emitA(t):
            r0, r1 = t * P, (t + 1) * P
            # ---- loads ----
            par = pool.tile([P, 300], F32, tag="par")
            for c in range(3):
                nc.sync.dma_start(par[:, 100 * c : 100 * (c + 1)], par_d[c][r0:r1, :])
            ang = pool.tile([P, NQ], F32, tag="ang")
            nc.sync.dma_start(ang[:], ang_d[r0:r1, :])
            dth = pool.tile([P, 1], F32, tag="dth")
            nc.sync.dma_start(dth[:], dth_d[r0:r1, :])

            # ---- trig (fp32) ----
            # ScalarE Sin covers [-pi, pi]; quarter angles:
            # u = sin(h/2), w = cos(h/2); sin(h)=2uw, cos(h)=1-2u^2.
            ch = pool.tile([P, 300], F32, tag="ch")
            sh = pool.tile([P, 300], F32, tag="sh")
            nsh = pool.tile([P, 300], F32, tag="nsh")
            trA = pool.tile([P, 100], F32, tag="trA")
            trB = pool.tile([P, 100], F32, tag="trB")

            def emit_trig(dst_s, dst_c, src, scale, scrA, scrB, n):
                bal.add("ACT", "ts_u", 2 * n)
                nc.scalar.activation(dst_s, src, ACTF.Sin, scale=scale)
                nc.scalar.activation(dst_c, src, ACTF.Sin, scale=scale, bias=pi2c)
                TT(scrA, dst_s, dst_c, AOT.mult, n, packed=False)
                TT(scrB, dst_s, dst_s, AOT.mult, n, packed=False)
                TS(dst_s, scrA, 2.0, n, packed=False)
                e = bal.pick(("DVE", "ACT", "POOL"), "ts_u", n)
                if e == "DVE":
                    nc.vector.tensor_scalar(dst_c, scrB, -2.0, 1.0, op0=AOT.mult, op1=AOT.add)
                elif e == "POOL":
                    nc.gpsimd.tensor_scalar(dst_c, scrB, -2.0, 1.0, op0=AOT.mult, op1=AOT.add)
                else:
                    nc.scalar.activation(dst_c, scrB, ACTF.Copy, scale=-2.0, bias=1.0)

            for c in range(3):
                src = par[:, 100 * c : 100 * (c + 1)]
                dst_s = sh[:, 100 * c : 100 * (c + 1)]
                dst_c = ch[:, 100 * c : 100 * (c + 1)]
                if c == DIAG:
                    emit_trig(dst_s, dst_c, src, 0.25, trA[:], trB[:], 100)
                else:
                    emit_trig(dst_s, dst_c, src, dth[:, 0:1], trA[:], trB[:], 100)
                    # fix CRX cols (30-49, 80-99): no dt factor
                    lx = lambda ap: ap.rearrange("p (l x) -> p l x", l=2, x=50)[:, :, 30:50]
                    emit_trig(
                        lx(dst_s), lx(dst_c), lx(src), 0.25,
                        trA[:, 0:40].rearrange("p (l x) -> p l x", l=2, x=20),
                        trB[:, 0:40].rearrange("p (l x) -> p l x", l=2, x=20),
                        40,
                    )
            TS(nsh[:], sh[:], -1.0, 300, packed=False)

            angc = pool.tile([P, NQ], F32, tag="angc")
            angs = pool.tile([P, NQ], F32, tag="angs")
            emit_trig(angs[:], angc[:], ang[:], 0.25, trA[:, 0:NQ], trB[:, 0:NQ], NQ)
            a3c = pool.tile([P, 30], F32, tag="a3c")
            a3s = pool.tile([P, 30], F32, tag="a3s")
            for dst, srcv in ((a3c, angc), (a3s, angs)):
                TS(dst[:, 0:10], srcv[:], 1.0, 10, packed=False)
                TS(dst[:, 10:20], srcv[:, ::-1], 1.0, 10, packed=False)
                TS(dst[:, 20:30], srcv[:], 1.0, 10, packed=False)

            # ---- u-coefficients per layer (fp32, (128,30)) ----
            ch3 = ch[:].rearrange("p (c x) -> p c x", c=3, x=100)
            sh3 = sh[:].rearrange("p (c x) -> p c x", c=3, x=100)
            m1 = pool.tile([P, 30], F32, tag="m1")
            m2 = pool.tile([P, 30], F32, tag="m2")
            m3 = pool.tile([P, 30], F32, tag="m3")
            m4 = pool.tile([P, 30], F32, tag="m4")
            w1 = pool.tile([P, 30], F32, tag="w1")
            w2 = pool.tile([P, 30], F32, tag="w2")
            V = lambda tl: tl[:].rearrange("p (c g) -> p c g", c=3, g=10)
            U = []
            for L in range(2):
                ca = ch3[:, :, 50 * L : 50 * L + 30 : 3]
                cb = ch3[:, :, 50 * L + 1 : 50 * L + 30 : 3]
                cg = ch3[:, :, 50 * L + 2 : 50 * L + 30 : 3]
                sa = sh3[:, :, 50 * L : 50 * L + 30 : 3]
                sb = sh3[:, :, 50 * L + 1 : 50 * L + 30 : 3]
                sg = sh3[:, :, 50 * L + 2 : 50 * L + 30 : 3]
                u = {
                    k: pool.tile([P, 30], F32, tag=f"u{k}{L}", name=f"u{k}{L}")
                    for k in ("p", "q", "nq", "r", "nr", "s", "ns")
                }
                TT(V(m1), cb, ca, AOT.mult, 30, packed=False)
                TT(V(m2), sb, sa, AOT.mult, 30, packed=False)
                TT(V(m3), sb, ca, AOT.mult, 30, packed=False)
                TT(V(m4), cb, sa, AOT.mult, 30, packed=False)
                TT(V(w1), cg, V(m1), AOT.mult, 30, packed=False)
                TT(V(w2), sg, V(m2), AOT.mult, 30, packed=False)
                TT(V(u["p"]), V(w1), V(w2), AOT.add, 30, packed=False)
                TT(V(w1), cg, V(m2), AOT.mult, 30, packed=False)
                TT(V(w2), sg, V(m1), AOT.mult, 30, packed=False)
                TT(V(u["q"]), V(w1), V(w2), AOT.subtract, 30, packed=False)
                TT(V(w1), cg, V(m3), AOT.mult, 30, packed=False)
                TT(V(w2), sg, V(m4), AOT.mult, 30, packed=False)
                TT(V(u["nr"]), V(w1), V(w2), AOT.add, 30, packed=False)
                TT(V(w1), sg, V(m3), AOT.mult, 30, packed=False)
                TT(V(w2), cg, V(m4), AOT.mult, 30, packed=False)
                TT(V(u["s"]), V(w1), V(w2), AOT.subtract, 30, packed=False)
                TS(u["nq"][:], u["q"][:], -1.0, 30, packed=False)
                TS(u["r"][:], u["nr"][:], -1.0, 30, packed=False)
                TS(u["ns"][:], u["s"][:], -1.0, 30, packed=False)
                U.append(u)

            # ---- v vectors: layer-0 rotations folded into init ----
            u0 = U[0]
            v0r = pool.tile([P, 30], F32, tag="v0r")
            v0i = pool.tile([P, 30], F32, tag="v0i")
            v1r = pool.tile([P, 30], F32, tag="v1r")
            v1i = pool.tile([P, 30], F32, tag="v1i")
            nv0i = pool.tile([P, 30], F32, tag="nv0i")
            nv1i = pool.tile([P, 30], F32, tag="nv1i")
            for dst, t1, t2 in (
                (v0r, ("p", a3c), ("r", a3s)),
                (v0i, ("q", a3c), ("s", a3s)),
                (v1r, ("nr", a3c), ("p", a3s)),
                (v1i, ("s", a3c), ("nq", a3s)),
            ):
                TT(w1[:], u0[t1[0]][:], t1[1][:], AOT.mult, 30, packed=False)
                TT(w2[:], u0[t2[0]][:], t2[1][:], AOT.mult, 30, packed=False)
                TT(dst[:], w1[:], w2[:], AOT.add, 30, packed=False)

            # fold combine coefficient c_k into wire col (10c) of each circuit's v
            # (linearity: c*psi = circuit applied to c*psi0). cf cols per c:
            # [cr, ci, nci].
            for c in range(3):
                j = 10 * c
                crc = cf_t[:, 3 * c : 3 * c + 1]
                cic = cf_t[:, 3 * c + 1 : 3 * c + 2]
                ncic = cf_t[:, 3 * c + 2 : 3 * c + 3]
                for vr, vi in ((v0r, v0i), (v1r, v1i)):
                    ax(w1[:, 0:1], vr[:, j : j + 1], crc, vi[:, j : j + 1], ncic, 1)
                    ax(w2[:, 0:1], vi[:, j : j + 1], crc, vr[:, j : j + 1], cic, 1)
                    TS(vr[:, j : j + 1], w1[:, 0:1], 1.0, 1, packed=False, engines=("DVE",))
                    TS(vi[:, j : j + 1], w2[:, 0:1], 1.0, 1, packed=False, engines=("DVE",))
            TS(nv0i[:], v0i[:], -1.0, 30, packed=False)
            TS(nv1i[:], v1i[:], -1.0, 30, packed=False)

            # ---- staircase w vectors: w = RX(theta_ring1[j-1]) v(g=j) ----
            # (p, 3, 9) grids: circuit-major columns.
            wt = {k: pool.tile([P, 27], F32, tag=f"wt{k}", name=f"wt{k}") for k in
                  ("w0r", "w0i", "w1r", "w1i", "nw0i", "nw1i")}
            W9 = lambda tl: tl[:].rearrange("p (c g) -> p c g", c=3, g=9)
            csv = ch3[:, :, 30:39]
            snv = sh3[:, :, 30:39]
            vv = lambda tl: V(tl)[:, :, 1:10]
            wm1 = pool.tile([P, 27], F32, tag="wm1")
            wm2 = pool.tile([P, 27], F32, tag="wm2")
            for dst, (f1a, f1b), (f2a, f2b), op in (
                ("w0r", (csv, v0r), (snv, v1i), AOT.add),      # c*v0r + sn*v1i
                ("w0i", (csv, v0i), (snv, v1r), AOT.subtract), # c*v0i - sn*v1r
                ("w1r", (snv, v0i), (csv, v1r), AOT.add),      # sn*v0i + c*v1r
                ("w1i", (csv, v1i), (snv, v0r), AOT.subtract), # c*v1i - sn*v0r
            ):
                TT(W9(wm1), f1a, vv(f1b), AOT.mult, 27, packed=False)
                TT(W9(wm2), f2a, vv(f2b), AOT.mult, 27, packed=False)
                TT(W9(wt[dst]), W9(wm1), W9(wm2), op, 27, packed=False)
            TS(wt["nw0i"][:], wt["w0i"][:], -1.0, 27, packed=False)
            TS(wt["nw1i"][:], wt["w1i"][:], -1.0, 27, packed=False)

            # ---- state build: staircase (init + ring1-L0) ----
            st = [bigpool.tile([P, 2 * DIM], F16, tag=f"st{c}", name=f"st{c}") for c in range(3)]
            PR = [bigpool.tile([P, 8192], F16, tag=f"PR{c}", name=f"PR{c}") for c in range(3)]
            ES = [PR[c][:, 0:2048] for c in range(3)]

            wcol = lambda k, c, g: wt[k][:, 9 * c + g : 9 * c + g + 1]

            # E_1 init: [v0(g=0), v1(g=0)] (fp32 -> fp16 convert copies)
            for c in range(3):
                j0 = 10 * c
                TS(ES[c][:, 0:1], col(v0r, j0), 1.0, 1, packed=False, engines=("DVE",))
                TS(ES[c][:, 1:2], col(v1r, j0), 1.0, 1, packed=False, engines=("DVE",))
                TS(ES[c][:, 1024:1025], col(v0i, j0), 1.0, 1, packed=False, engines=("DVE",))
                TS(ES[c][:, 1025:1026], col(v1i, j0), 1.0, 1, packed=False, engines=("DVE",))

            # staircase steps j=1..9: extend with wire g=j, gate CRX(ring1[j-1])
            # FWD/DIAG: new dim innermost; BWD: new dim outermost.
            # buffers: E_j in ES for odd j... E_j lives in ES if j odd else st.
            for j in range(1, NQ):
                W = 1 << j  # current complex width
                for c in range(3):
                    src_b = ES[c] if (j % 2 == 1) else st[c]
                    dst_b = st[c] if (j % 2 == 1) else ES[c]
                    jc = 10 * c + j
                    inner = c in (FWD, DIAG)
                    for cb in (0, 1):
                        if cb == 0:
                            ur = [col(v0r, jc), col(v1r, jc)]
                            ui = [col(v0i, jc), col(v1i, jc)]
                            nui = [col(nv0i, jc), col(nv1i, jc)]
                        else:
                            ur = [wcol("w0r", c, j - 1), wcol("w1r", c, j - 1)]
                            ui = [wcol("w0i", c, j - 1), wcol("w1i", c, j - 1)]
                            nui = [wcol("nw0i", c, j - 1), wcol("nw1i", c, j - 1)]
                        for tt in (0, 1):
                            for pl in (0, 1):  # 0: re-out, 1: im-out
                                po = 1024 * pl
                                if inner:
                                    sv = lambda plane: (
                                        src_b[:, 1024 * plane : 1024 * plane + W]
                                        .rearrange("p (x c2) -> p x c2", x=W // 2, c2=2)[:, :, cb]
                                    )
                                    dv = (
                                        dst_b[:, po : po + 2 * W]
                                        .rearrange("p (x c2 t2) -> p x c2 t2", x=W // 2, c2=2, t2=2)
                                        [:, :, cb, tt]
                                    )
                                else:
                                    sv = lambda plane: src_b[
                                        :, 1024 * plane + cb * (W // 2) : 1024 * plane + (cb + 1) * (W // 2)
                                    ]
                                    dv = dst_b[
                                        :, po + tt * W + cb * (W // 2) : po + tt * W + (cb + 1) * (W // 2)
                                    ]
                                if pl == 0:
                                    ax(dv, sv(0), ur[tt], sv(1), nui[tt], W // 2)
                                else:
                                    ax(dv, sv(1), ur[tt], sv(0), ui[tt], W // 2)
            # E_10 lands in st (j=9 odd -> dst st) for all circuits.
            return ch, sh, nsh, U, st, PR

        def emitB(t, ch, sh, nsh, U, st, PR):
            r0, r1 = t * P, (t + 1) * P
            # ---- full-state gate emitters ----
            def emit_rot(c, w, u, j):
                """SU(2) rotation on wire w; coeffs u[...][:, j]."""
                sp = col(u["p"], j); sq = col(u["q"], j); snq = col(u["nq"], j)
                sr = col(u["r"], j); snr = col(u["nr"], j)
                ss = col(u["s"], j); sns = col(u["ns"], j)
                stc = st[c]
                pr = PR[c]
                if w <= 8:
                    inner = 1 << (9 - w)
                    m = 1 << (w + 1)
                    fv = stc[:].rearrange("p (m t i) -> p m t i", m=m, t=2, i=inner)
                    gv = stc[:].rearrange(
                        "p (pl o t i) -> p pl o t i", pl=2, o=m // 2, t=2, i=inner
                    )
                    PA = pr[:, 0:2048].rearrange("p (m t i) -> p m t i", m=m, t=2, i=inner)
                    PB = pr[:, 2048:4096].rearrange("p (m t i) -> p m t i", m=m, t=2, i=inner)
                    pqv = lambda base: pr[:, base : base + 2048].rearrange(
                        "p (pl o t i) -> p pl o t i", pl=2, o=m // 2, t=2, i=inner
                    )
                    PC = pqv(4096)
                    PD = pqv(6144)
                    pk = inner >= 2
                    TS(PA, fv, sp, 2048, packed=pk)
                    TS(PB[:, :, 0, :], fv[:, :, 1, :], sr, 1024, packed=pk)
                    TS(PB[:, :, 1, :], fv[:, :, 0, :], snr, 1024, packed=pk)
                    TS(PC[:, 0, :, 0, :], gv[:, 1, :, 0, :], snq, 512, packed=pk)
                    TS(PC[:, 1, :, 0, :], gv[:, 0, :, 0, :], sq, 512, packed=pk)
                    TS(PC[:, 0, :, 1, :], gv[:, 1, :, 1, :], sq, 512, packed=pk)
                    TS(PC[:, 1, :, 1, :], gv[:, 0, :, 1, :], snq, 512, packed=pk)
                    TS(PD[:, 0, :, 0, :], gv[:, 1, :, 1, :], sns, 512, packed=pk)
                    TS(PD[:, 1, :, 0, :], gv[:, 0, :, 1, :], ss, 512, packed=pk)
                    TS(PD[:, 0, :, 1, :], gv[:, 1, :, 0, :], sns, 512, packed=pk)
                    TS(PD[:, 1, :, 1, :], gv[:, 0, :, 0, :], ss, 512, packed=pk)
                    TT(pr[:, 0:2048], pr[:, 0:2048], pr[:, 2048:4096], AOT.add, 2048)
                    TT(pr[:, 4096:6144], pr[:, 4096:6144], pr[:, 6144:8192], AOT.add, 2048)
                    PCm = pr[:, 4096:6144].rearrange(
                        "p (m t i) -> p m t i", m=m, t=2, i=inner
                    )
                    TT(fv, PA, PCm, AOT.add, 2048, packed=pk)
                else:  # w == 9: pairs are adjacent elements
                    fv = stc[:].rearrange("p (m t) -> p m t", m=1024, t=2)
                    gv = stc[:].rearrange("p (pl o t) -> p pl o t", pl=2, o=512, t=2)
                    PA = pr[:, 0:2048].rearrange("p (m t) -> p m t", m=1024, t=2)
                    PB = pr[:, 2048:4096].rearrange("p (m t) -> p m t", m=1024, t=2)
                    pqv = lambda base: pr[:, base : base + 2048].rearrange(
                        "p (pl o t) -> p pl o t", pl=2, o=512, t=2
                    )
                    PC = pqv(4096)
                    PD = pqv(6144)
                    TS(pr[:, 0:2048], stc[:], sp, 2048)
                    TS(PB[:, :, 0], fv[:, :, 1], sr, 1024, packed=False)
                    TS(PB[:, :, 1], fv[:, :, 0], snr, 1024, packed=False)
                    TS(PC[:, 0, :, 0], gv[:, 1, :, 0], snq, 512, packed=False)
                    TS(PC[:, 1, :, 0], gv[:, 0, :, 0], sq, 512, packed=False)
                    TS(PC[:, 0, :, 1], gv[:, 1, :, 1], sq, 512, packed=False)
                    TS(PC[:, 1, :, 1], gv[:, 0, :, 1], snq, 512, packed=False)
                    TS(PD[:, 0, :, 0], gv[:, 1, :, 1], sns, 512, packed=False)
                    TS(PD[:, 1, :, 0], gv[:, 0, :, 1], ss, 512, packed=False)
                    TS(PD[:, 0, :, 1], gv[:, 1, :, 0], sns, 512, packed=False)
                    TS(PD[:, 1, :, 1], gv[:, 0, :, 0], ss, 512, packed=False)
                    TT(pr[:, 0:2048], pr[:, 0:2048], pr[:, 2048:4096], AOT.add, 2048)
                    TT(pr[:, 4096:6144], pr[:, 4096:6144], pr[:, 6144:8192], AOT.add, 2048)
                    TT(stc[:], pr[:, 0:2048], pr[:, 4096:6144], AOT.add, 2048)

            def emit_crx(c, ctrl, tgt, cl):
                cc = col(ch, cl)
                ssc = col(sh, cl)
                nsc = col(nsh, cl)
                stc = st[c]
                pr = PR[c]
                hi, lo = min(ctrl, tgt), max(ctrl, tgt)
                PQ = pr[:, 0:1024]
                PS = pr[:, 1024:2048]
                if lo - hi == 1:
                    a = 1 << hi
                    z = 1 << (8 - hi)
                    v6 = stc[:].rearrange(
                        "p (pl a x y z) -> p pl a x y z", pl=2, a=a, x=2, y=2, z=z
                    )
                    vm = stc[:].rearrange(
                        "p (pla x y z) -> p pla x y z", pla=2 * a, x=2, y=2, z=z
                    )
                    PSm = PS.rearrange("p (pla y z) -> p pla y z", pla=2 * a, y=2, z=z)
                    if ctrl < tgt:
                        # ctrl bit = x, tgt bit = y
                        Q = vm[:, :, 1, :, :]                     # (p, pla, y, z)
                        Qr = lambda pl: v6[:, pl, :, 1, ::-1, :]  # (p, a, y, z) tgt-rev
                        QPv = PQ.rearrange("p (pla y z) -> p pla y z", pla=2 * a, y=2, z=z)
                        PSv = PS.rearrange("p (pl a y z) -> p pl a y z", pl=2, a=a, y=2, z=z)
                        TS(QPv, Q, cc, 1024)
                        # s-products, tgt-swapped, sign per plane
                        TS(PSv[:, 0, :, :, :], Qr(1), ssc, 512)
                        TS(PSv[:, 1, :, :, :], Qr(0), nsc, 512)
                        TT(Q, QPv, PSm, AOT.add, 1024)
                    else:
                        # tgt bit = x, ctrl bit = y
                        Q = vm[:, :, :, 1, :]                     # (p, pla, x, z)
                        Qr = lambda pl: v6[:, pl, :, ::-1, 1, :]
                        QPv = PQ.rearrange("p (pla x z) -> p pla x z", pla=2 * a, x=2, z=z)
                        PSv = PS.rearrange("p (pl a x z) -> p pl a x z", pl=2, a=a, x=2, z=z)
                        TS(QPv, Q, cc, 1024)
                        TS(PSv[:, 0, :, :, :], Qr(1), ssc, 512)
                        TS(PSv[:, 1, :, :, :], Qr(0), nsc, 512)
                        TT(Q, QPv, PSm, AOT.add, 1024)
                else:
                    raise AssertionError("wrap pair handled by emit_crx_wrap")

            def emit_crx_wrap(c, ctrl, tgt, cl):
                cc = col(ch, cl)
                ssc = col(sh, cl)
                nsc = col(nsh, cl)
                stc = st[c]
                pr = PR[c]
                PQ = pr[:, 0:1024]
                PS = pr[:, 1024:2048]
                v5 = stc[:].rearrange(
                    "p (pl x mid y) -> p pl x mid y", pl=2, x=2, mid=256, y=2
                )
                if ctrl == 0:
                    # ctrl = x (stride 512), tgt = y (stride 1): quarter x=1
                    Q = v5[:, :, 1, :, :]                      # (p, pl, mid, y)
                    Qp = lambda pl, y: v5[:, pl, 1, :, y]      # (p, mid) stride 2
                    QPv = PQ.rearrange("p (pl mid y) -> p pl mid y", pl=2, mid=256, y=2)
                    PSv = PS.rearrange("p (pl mid y) -> p pl mid y", pl=2, mid=256, y=2)
                    TS(QPv, Q, cc, 1024)
                    TS(PSv[:, 0, :, :], v5[:, 1, 1, :, ::-1], ssc, 512)
                    TS(PSv[:, 1, :, :], v5[:, 0, 1, :, ::-1], nsc, 512)
                    TT(Q, QPv, PSv, AOT.add, 1024)
                else:
                    # ctrl = 9 (y, stride 1), tgt = 0 (x, stride 512): quarter y=1
                    Q = v5[:, :, :, :, 1]                      # (p, pl, x, mid)
                    Qp = lambda pl, x: v5[:, pl, x, :, 1]      # (p, mid) stride 2
                    QPv = PQ.rearrange("p (pl x mid) -> p pl x mid", pl=2, x=2, mid=256)
                    PSv = PS.rearrange("p (pl x mid) -> p pl x mid", pl=2, x=2, mid=256)
                    TS(QPv, Q, cc, 1024, packed=False)
                    TS(PSv[:, 0, :, :], v5[:, 1, ::-1, :, 1], ssc, 512, packed=False)
                    TS(PSv[:, 1, :, :], v5[:, 0, ::-1, :, 1], nsc, 512, packed=False)
                    TT(Q, QPv, PSv, AOT.add, 1024, packed=False)

            def emit_crx_any(c, ctrl, tgt, cl):
                if abs(ctrl - tgt) == 1:
                    emit_crx(c, ctrl, tgt, cl)
                else:
                    emit_crx_wrap(c, ctrl, tgt, cl)

            # ---- gate sequences (interleave circuits for engine overlap) ----
            seqs = []
            for c in range(3):
                g = []
                ring0 = _ring_gates(c, 0)
                g.append(("crx", ring0[9]))            # ring1-L0 wrap gate
                for e in ring0[10:20]:
                    g.append(("crx", e))               # ring2-L0
                for gg in range(NQ):                   # rotations L1 (commuting)
                    w = gg if c != BWD else 9 - gg
                    g.append(("rot", (w, 10 * c + gg)))
                for e in _ring_gates(c, 1):
                    g.append(("crx", e))               # ring1-L1 + ring2-L1
                seqs.append(g)
            for step in range(len(seqs[0])):
                for c in range(3):
                    kind, arg = seqs[c][step]
                    if kind == "crx":
                        ctrl, tgt, cl = arg
                        emit_crx_any(c, ctrl, tgt, cl)
                    else:
                        w, j = arg
                        emit_rot(c, w, U[1], j)

            # ---- combine: st0 += st1 + st2 (coeffs already folded) ----
            TT(st[0][:], st[0][:], st[1][:], AOT.add, 2048)
            TT(st[0][:], st[0][:], st[2][:], AOT.add, 2048)

            # ---- measure ----
            acc = st[0]
            Rt = bigpool.tile([P, 2048], F16, tag="Rt")   # R = -i * acc
            crX = pool.tile([P, NQ], F32, tag="crX")
            crY = pool.tile([P, NQ], F32, tag="crY")
            hZ = pool.tile([P, NQ], F32, tag="hZ")
            scol = pool.tile([P, 8], F32, tag="scol")
            TS(Rt[:, 0:1024], acc[:, 1024:2048], 1.0, 1024)
            TS(Rt[:, 1024:2048], acc[:, 0:1024], -1.0, 1024)

            # distinct dummy slots per accum so WAW doesn't serialize them
            dXs = lambda w: PR[0][:, 1024 * (w % 8) : 1024 * (w % 8) + 1024]
            dYs = lambda w: PR[1][:, 1024 * (w % 8) : 1024 * (w % 8) + 1024]
            dZs = lambda w: PR[2][:, 1024 * (w % 8) : 1024 * (w % 8) + 1024]
            SQACC(PR[2][:, 0:2048], acc[:], scol[:, 0:1], 2048)
            for w in range(NQ):
                if w <= 8:
                    inner = 1 << (9 - w)
                    m = 1 << (w + 1)
                    fv = acc[:].rearrange("p (m t i) -> p m t i", m=m, t=2, i=inner)
                    rv = Rt[:].rearrange("p (m t i) -> p m t i", m=m, t=2, i=inner)
                    a0 = fv[:, :, 0, :]
                    a1 = fv[:, :, 1, :]
                    rr1 = rv[:, :, 1, :]
                    shp = lambda d: d.rearrange("p (m i) -> p m i", m=m, i=inner)
                else:
                    fv = acc[:].rearrange("p (m t) -> p m t", m=1024, t=2)
                    rv = Rt[:].rearrange("p (m t) -> p m t", m=1024, t=2)
                    a0 = fv[:, :, 0]
                    a1 = fv[:, :, 1]
                    rr1 = rv[:, :, 1]
                    shp = lambda d: d
                ACC(shp(dXs(w)), a0, a1, crX[:, w : w + 1], 1024)
                ACC(shp(dYs(w)), a0, rr1, crY[:, w : w + 1], 1024)
                SQACC(shp(dZs(w + 2)), a1, hZ[:, w : w + 1], 1024)

            # inv = 1/(S + 1e-9); s1 = 2*inv; s2 = -2*inv; sz = S*inv
            nc.vector.tensor_scalar(scol[:, 1:2], scol[:, 0:1], 1e-9, None, op0=AOT.add)
            nc.vector.reciprocal(scol[:, 2:3], scol[:, 1:2])
            nc.vector.tensor_scalar(scol[:, 3:4], scol[:, 2:3], 2.0, None, op0=AOT.mult)
            nc.vector.tensor_scalar(scol[:, 4:5], scol[:, 2:3], -2.0, None, op0=AOT.mult)
            ttd(scol[:, 5:6], scol[:, 0:1], scol[:, 2:3], op=AOT.mult)

            out30 = pool.tile([P, 30], F32, tag="out30")
            nc.vector.tensor_scalar_mul(out30[:, 0:10], crX[:], scol[:, 3:4])
            nc.vector.tensor_scalar_mul(out30[:, 10:20], crY[:], scol[:, 3:4])
            szb = scol[:, 5:6].broadcast_to([P, 1, NQ])
            nc.vector.scalar_tensor_tensor(
                out30[:, 20:30].unsqueeze(1), hZ[:].unsqueeze(1), scol[:, 4:5], szb,
                op0=AOT.mult, op1=AOT.add,
            )
            nc.sync.dma_start(out_d[r0:r1, :], out30[:])

        cx = emitA(0)
        for t in range(n_tiles):
            nx = emitA(t + 1) if t + 1 < n_tiles else None
            emitB(t, *cx)
            cx = nx


def build_nc(n_tiles=NT, b_core=None):
    if b_core is None:
        b_core = n_tiles * P
    nc = bacc.Bacc("TRN2", target_bir_lowering=False)
    ins = {
        "input_angles": nc.dram_tensor("input_angles", [b_core, NQ], F32, kind="ExternalInput")[:],
        "forward_params": nc.dram_tensor("forward_params", [b_core, 100], F32, kind="ExternalInput")[:],
        "backward_params": nc.dram_tensor("backward_params", [b_core, 100], F32, kind="ExternalInput")[:],
        "diagonal_params": nc.dram_tensor("diagonal_params", [b_core, 100], F32, kind="ExternalInput")[:],
        "dth": nc.dram_tensor("dth", [b_core, 1], F32, kind="ExternalInput")[:],
        "cf": nc.dram_tensor("cf", [P, 9], F32, kind="ExternalInput")[:],
    }
    outs = {"out": nc.dram_tensor("out", [b_core, 30], F32, kind="ExternalOutput")[:]}
    with tile.TileContext(nc) as tc:
        emit_core_kernel(nc, tc, ins, outs, n_tiles=n_tiles)
    nc.compile()
    return nc


_NC_CACHE = {}


def _get_nc(n_tiles=NT):
    if n_tiles not in _NC_CACHE:
        _NC_CACHE[n_tiles] = build_nc(n_tiles)
    return _NC_CACHE[n_tiles]


def make_host_inputs(input_angles, forward_params, backward_params, diagonal_params,
                     dt_scale, alpha_real, alpha_imag, beta_real, beta_imag,
                     gamma_real, gamma_imag):
    """Host-side scalar prep shared by kernel() and tests."""
    al = complex(float(alpha_real), float(alpha_imag))
    be = complex(float(beta_real), float(beta_imag))
    ga = complex(float(gamma_real), float(gamma_imag))
    n = np.sqrt(abs(al) ** 2 + abs(be) ** 2 + abs(ga) ** 2 + 1e-9)
    cs = [al / n, be / n, ga / n]
    row = []
    for ck in cs:
        row += [ck.real, ck.imag, -ck.imag]
    cf = np.tile(np.asarray(row, np.float32), (P, 1))
    dth = (0.25 * np.asarray(dt_scale, np.float32)).reshape(-1, 1)
    return cf, dth


def kernel(**inputs):
    from concourse.bass_utils import run_bass_kernel_spmd

    cf, dth = make_host_inputs(**inputs)
    ang = np.ascontiguousarray(np.asarray(inputs["input_angles"], np.float32))
    pf = np.ascontiguousarray(np.asarray(inputs["forward_params"], np.float32))
    pb = np.ascontiguousarray(np.asarray(inputs["backward_params"], np.float32))
    pd = np.ascontiguousarray(np.asarray(inputs["diagonal_params"], np.float32))

    nc = _get_nc(NT)
    in_maps = []
    for c in range(N_CORES):
        r0, r1 = c * B_CORE, (c + 1) * B_CORE
        in_maps.append({
            "input_angles": ang[r0:r1],
            "forward_params": pf[r0:r1],
            "backward_params": pb[r0:r1],
            "diagonal_params": pd[r0:r1],
            "dth": np.ascontiguousarray(dth[r0:r1]),
            "cf": cf,
        })
    res = run_bass_kernel_spmd(nc, in_maps, core_ids=list(range(N_CORES)))
    out = np.concatenate([res.results[c]["out"] for c in range(N_CORES)], axis=0)
    return out.astype(np.float32)
